# revision 1
# baseline (speedup 1.0000x reference)
"""TRN2 Bass kernel for nn_LocalPoolPointnetPPFusion (batch-parallel, 8 cores).

Per-core pipeline (feature-major activations [128, 8192] bf16, biases deferred):
  net0' = p @ wp (+ p2 @ wp2 for corr stream)         (biases deferred to host)
  5 resblocks per stream (in-place, bf16 matmuls, fp32 PSUM); between blocks:
    net_fm --xbar--> net_pm [128, 65, 128] (chunk 64 = zeros, stays in SBUF)
    per plane: SBUF-source transpose-gathers build occupancy-sorted FM strips,
    prefix TT-max -> per-bin max (FM) --xbar--> table_pm -> SBUF-source
    transpose-gather expands to pooled' FM; 3 planes summed.
  final stage: same strips with fp32 prefix TT-add (per-bin sums of net'),
    cast bf16, @ fc_w on PE -> PM fp32 -> collision-free dma_scatter_add into
    zero-donated output grids [R*R, C] (HBM only here).
  host folds all deferred biases + fc bias + 1/cnt + transposes to [C, R, R].
"""
import sys
sys.path.insert(0, "/opt/trn_rl_repo")

import numpy as np
import ml_dtypes

BF = ml_dtypes.bfloat16
F32 = np.float32

B, T, H, C, R = 8, 8192, 128, 128, 128
NB = 5
NPLANES = 3
PLANE_COLS = ((0, 2), (0, 1), (1, 2))
ZROW = T          # zero-token index (chunk 64 of net_pm)
CHUNK = 2048      # mean-stage slot chunk
SCHUNK = 1024     # scatter chunk (PM sums tile)


def compute_idx_lists(p_np):
    import jax
    import jax.numpy as jnp
    cpu = jax.devices("cpu")[0]
    out = []
    with jax.default_device(cpu):
        pj = jnp.asarray(p_np)
        for cols in PLANE_COLS:
            xy = pj[..., jnp.array(cols)] / (1.0 + 0.0 + 1e-3) + 0.5
            xy = jnp.clip(xy, 0.0, 1.0 - 1e-3)
            g = jnp.floor(xy * R).astype(jnp.int32)
            out.append(np.asarray(g[..., 0] + R * g[..., 1]))
    return out


def wrap_idxs(flat):
    """token i -> idxs[i%16, i//16]; replicated to 128 partitions."""
    flat = np.asarray(flat, np.int64)
    n = len(flat)
    assert n % 16 == 0
    a = np.zeros((16, n // 16), np.int16)
    for i in range(n):
        a[i % 16, i // 16] = flat[i]
    return np.tile(a, (8, 1))


def ceil128(x):
    return max((int(x) + 127) // 128 * 128, 128)


class PlanePrep:
    def __init__(self, idx):
        self.idx = idx
        cnt = np.bincount(idx, minlength=R * R)
        self.cnt = cnt
        occ = np.where(cnt > 0)[0]
        order = np.argsort(-cnt[occ], kind="stable")
        self.bins_sorted = occ[order]
        self.n_occ = len(occ)
        self.occ_sorted = cnt[self.bins_sorted]
        sort_by_bin = np.argsort(idx, kind="stable")
        starts = np.searchsorted(idx[sort_by_bin], self.bins_sorted)
        self.members = [sort_by_bin[s:s + k] for s, k in zip(starts, self.occ_sorted)]
        slot_of_bin = np.full(R * R, -1, np.int64)
        slot_of_bin[self.bins_sorted] = np.arange(self.n_occ)
        self.pidx = slot_of_bin[idx]
        self.R_max = int(self.occ_sorted[0])
        self.n_r = [int((self.occ_sorted >= r).sum()) for r in range(1, self.R_max + 1)]

    def nr(self, r):
        return self.n_r[r - 1] if r <= self.R_max else 0

    def round_ids(self, r, width, sum_pad):
        ids = np.full(width, ZROW, np.int64)
        nr = self.nr(r)
        for s in range(min(nr, width)):
            ids[s] = self.members[s][r - 1]
        if not sum_pad:
            for s in range(nr, width):
                ids[s] = self.members[s][0] if s < self.n_occ else ZROW
        return ids


def _build(inputs, preps, REPS=1, timing=False):
    """Build program + per-core in_maps. timing=True uses internal grids."""
    import concourse.bacc as bacc
    import concourse.tile as tile
    from concourse import mybir

    p = np.asarray(inputs["p"], F32)
    p2 = np.asarray(inputs["p2"], F32)

    N1P = [max(ceil128(preps[b][pl].n_occ) for b in range(B)) for pl in range(NPLANES)]
    RMAX = [max(preps[b][pl].R_max for b in range(B)) for pl in range(NPLANES)]
    CR = []
    for pl in range(NPLANES):
        CR.append([ceil128(max(preps[b][pl].nr(r) for b in range(B)))
                   for r in range(2, RMAX[pl] + 1)])
    MAXCR = max(max(c) if c else 128 for c in CR)
    MAXN1P = max(N1P)

    def stream_host(pref, base_bias):
        w0 = np.asarray(inputs[f"{pref}_w0"], F32)
        b0 = np.asarray(inputs[f"{pref}_b0"], F32)
        w1 = np.asarray(inputs[f"{pref}_w1"], F32)
        b1 = np.asarray(inputs[f"{pref}_b1"], F32)
        ws = np.asarray(inputs[f"{pref}_ws"], F32)
        relu_bias = []
        Bp = base_bias
        for i in range(NB):
            if i == 0:
                bias_in = Bp
                relu_bias.append((bias_in[:H].copy(), bias_in[H:].copy()))
            else:
                bias_in = np.concatenate([Bp, 3.0 * Bp])
                relu_bias.append((Bp.copy(), 3.0 * Bp))
            Bp = b1[i] + bias_in @ ws[i]
        return dict(w0=w0, b0=b0, w1=w1, ws=ws, relu_bias=relu_bias, B_final=Bp)

    wp = np.asarray(inputs["wp"], F32)
    bp = np.asarray(inputs["bp"], F32)
    wp2 = np.asarray(inputs["wp2"], F32)
    bp2 = np.asarray(inputs["bp2"], F32)
    sh_host = {"g": stream_host("blk", bp.copy()), "c": stream_host("blkc", bp + bp2)}
    fc_w = {"g": np.asarray(inputs["fc_c_w"], F32),
            "c": np.asarray(inputs["fc_cc_w"], F32)}
    fc_b = {"g": np.asarray(inputs["fc_c_b"], F32),
            "c": np.asarray(inputs["fc_cc_b"], F32)}
    cvec = {s: sh_host[s]["B_final"] @ fc_w[s] + fc_b[s] for s in ("g", "c")}

    nc = bacc.Bacc("TRN2", target_bir_lowering=False, debug=False, num_devices=B)
    dt = mybir.dt

    def din(name, shape, dtype):
        return nc.dram_tensor(name, shape, dtype, kind="ExternalInput")

    pT_d = din("pT", [3, T], dt.bfloat16)
    p2T_d = din("p2T", [3, T], dt.bfloat16)
    wp_d = din("wp", [3, 2 * H], dt.bfloat16)
    wp2_d = din("wp2", [3, 2 * H], dt.bfloat16)
    wpk_d = {}
    for s in ("g", "c"):
        wpk_d[s] = dict(
            w0=din(f"{s}_w0", [H, NB, 2 * H], dt.bfloat16),
            w1=din(f"{s}_w1", [H, NB, H], dt.bfloat16),
            ws=din(f"{s}_ws", [H, NB, 2 * H], dt.bfloat16),
            rb=din(f"{s}_rb", [H, NB, 2], dt.float32),
            b0=din(f"{s}_b0", [H, NB], dt.float32),
            fcw=din(f"{s}_fcw", [H, C], dt.bfloat16),
        )
    g1_d = [din(f"g1_{pl}", [128, N1P[pl] // 16], dt.int16) for pl in range(NPLANES)]
    gmax_d = [[din(f"gmax_{pl}_{r}", [128, CR[pl][r - 2] // 16], dt.int16)
               for r in range(2, RMAX[pl] + 1)] for pl in range(NPLANES)]
    gsum_d = [[din(f"gsum_{pl}_{r}", [128, CR[pl][r - 2] // 16], dt.int16)
               for r in range(2, RMAX[pl] + 1)] for pl in range(NPLANES)]
    pidx_d = [din(f"pidx_{pl}", [128, T // 16], dt.int16) for pl in range(NPLANES)]
    sbin_d = [din(f"sbin_{pl}", [128, N1P[pl] // 16], dt.int16) for pl in range(NPLANES)]

    out_kind = "Internal" if timing else "ExternalOutput"
    out_d = {(s, pl): nc.dram_tensor(f"out_{s}{pl}", [R * R, C], dt.float32,
                                     kind=out_kind)
             for s in ("g", "c") for pl in range(NPLANES)}
    chk_d = nc.dram_tensor("chk", [128, 128], dt.bfloat16, kind="ExternalOutput") \
        if timing else None

    with tile.TileContext(nc) as tc:
        with tc.tile_pool(name="const", bufs=1) as constp, \
             tc.tile_pool(name="act", bufs=1) as actp, \
             tc.tile_pool(name="pooledp", bufs=2) as pooledp, \
             tc.tile_pool(name="small", bufs=3) as smallp, \
             tc.tile_pool(name="sr", bufs=3) as srp, \
             tc.tile_pool(name="gp", bufs=1) as gp, \
             tc.tile_pool(name="npm", bufs=2) as npmp, \
             tc.tile_pool(name="pm", bufs=2) as pmp, \
             tc.tile_pool(name="psum", bufs=2, space="PSUM") as psump:

            wp_t = constp.tile([3, 2 * H], dt.bfloat16)
            wp2_t = constp.tile([3, 2 * H], dt.bfloat16)
            nc.sync.dma_start(wp_t[:], wp_d[:])
            nc.sync.dma_start(wp2_t[:], wp2_d[:])
            W = {}
            for s in ("g", "c"):
                W[s] = dict(
                    w0=constp.tile([H, NB, 2 * H], dt.bfloat16, tag=f"{s}w0", name=f"{s}w0"),
                    w1=constp.tile([H, NB, H], dt.bfloat16, tag=f"{s}w1", name=f"{s}w1"),
                    ws=constp.tile([H, NB, 2 * H], dt.bfloat16, tag=f"{s}ws", name=f"{s}ws"),
                    rb=constp.tile([H, NB, 2], dt.float32, tag=f"{s}rb", name=f"{s}rb"),
                    b0=constp.tile([H, NB], dt.float32, tag=f"{s}b0", name=f"{s}b0"),
                    fcw=constp.tile([H, C], dt.bfloat16, tag=f"{s}fcw", name=f"{s}fcw"),
                )
                for k, t in W[s].items():
                    nc.sync.dma_start(t[:], wpk_d[s][k][:])
            g1_t, gmax_t, gsum_t, pidx_t, sbin_t = [], [], [], [], []
            for pl in range(NPLANES):
                g1_t.append(constp.tile([128, N1P[pl] // 16], dt.int16,
                                        tag=f"g1{pl}", name=f"g1t{pl}"))
                pidx_t.append(constp.tile([128, T // 16], dt.int16,
                                          tag=f"pi{pl}", name=f"pit{pl}"))
                sbin_t.append(constp.tile([128, N1P[pl] // 16], dt.int16,
                                          tag=f"sb{pl}", name=f"sbt{pl}"))
                nc.sync.dma_start(g1_t[pl][:], g1_d[pl][:])
                nc.sync.dma_start(pidx_t[pl][:], pidx_d[pl][:])
                nc.sync.dma_start(sbin_t[pl][:], sbin_d[pl][:])
                gm, gs = [], []
                for j in range(RMAX[pl] - 1):
                    tm = constp.tile([128, CR[pl][j] // 16], dt.int16,
                                     tag=f"gm{pl}_{j}", name=f"gmt{pl}_{j}")
                    ts_ = constp.tile([128, CR[pl][j] // 16], dt.int16,
                                      tag=f"gs{pl}_{j}", name=f"gst{pl}_{j}")
                    nc.sync.dma_start(tm[:], gmax_d[pl][j][:])
                    nc.sync.dma_start(ts_[:], gsum_d[pl][j][:])
                    gm.append(tm)
                    gs.append(ts_)
                gmax_t.append(gm)
                gsum_t.append(gs)

            def sbuf_gather(dst_ap, src_pm, idxs_ap, n):
                """SBUF-source transpose gather: token i at [i%128, i//128, :]."""
                nc.gpsimd.dma_gather(
                    dst_ap, src_pm, idxs_ap, n, n, H,
                    transpose=True, single_packet=False,
                    sbuf_tokens_per_rank=128,
                    sbuf_free_dim_per_rank=H * 2,
                )

            def make_net_pm(s, net_fm):
                """Transpose net' into PM [128, 65, 128]; chunk 64 = zeros."""
                npm = npmp.tile([128, 65, H], dt.bfloat16, tag="npm", name="npm")
                nc.vector.memset(npm[:, 64, :], 0.0)
                nc.sync.dma_start_transpose(npm[:, :64, :], net_fm[:])
                return npm

            def pool_local(s, npm):
                pooled = pooledp.tile([H, T], dt.bfloat16, tag="pooled", name="pooled")
                for pl in range(NPLANES):
                    n1 = N1P[pl]
                    s1 = pmp.tile([128, 1, MAXN1P], dt.bfloat16, tag="pm", name="s1")
                    sbuf_gather(s1[:, :, :n1], npm[:], g1_t[pl][:], n1)
                    for j in range(RMAX[pl] - 1):
                        w = CR[pl][j]
                        sr = srp.tile([128, 1, MAXCR], dt.bfloat16, tag="sr", name="sr")
                        sbuf_gather(sr[:, :, :w], npm[:], gmax_t[pl][j][:], w)
                        nc.vector.tensor_tensor(
                            out=s1[:, 0, :w], in0=s1[:, 0, :w],
                            in1=sr[:, 0, :w], op=mybir.AluOpType.max)
                    tbl = pmp.tile([128, MAXN1P // 128, H], dt.bfloat16,
                                   tag="pm", name="tbl")
                    nc.sync.dma_start_transpose(tbl[:, :n1 // 128, :], s1[:, 0, :n1])
                    if pl == 0:
                        sbuf_gather(pooled[:].rearrange("h (a t) -> h a t", a=1),
                                    tbl[:], pidx_t[pl][:], T)
                    else:
                        g = gp.tile([128, 1, T], dt.bfloat16, tag="g", name="g")
                        sbuf_gather(g[:], tbl[:], pidx_t[pl][:], T)
                        nc.vector.tensor_tensor(out=pooled[:], in0=pooled[:],
                                                in1=g[:, 0, :], op=mybir.AluOpType.add)
                return pooled

            def resblock(s, i, xa, xb):
                """In-place: writes output into xa. Returns xa."""
                w = W[s]
                ba_ap = w["rb"][:, i, 0:1]
                bb_ap = w["rb"][:, i, 1:2]
                for nt in range(T // 512):
                    sl = slice(nt * 512, (nt + 1) * 512)
                    ra = smallp.tile([H, 512], dt.bfloat16, tag="ra", name="ra")
                    rb_ = smallp.tile([H, 512], dt.bfloat16, tag="rb", name="rb")
                    nc.vector.tensor_scalar(out=ra[:], in0=xa[:, sl], scalar1=ba_ap,
                                            scalar2=0.0, op0=mybir.AluOpType.add,
                                            op1=mybir.AluOpType.max)
                    nc.vector.tensor_scalar(out=rb_[:], in0=xb[:, sl], scalar1=bb_ap,
                                            scalar2=0.0, op0=mybir.AluOpType.add,
                                            op1=mybir.AluOpType.max)
                    ph = psump.tile([H, 512], dt.float32, tag="ph", name="ph")
                    nc.tensor.matmul(ph[:], w["w0"][:, i, :H], ra[:],
                                     start=True, stop=False)
                    nc.tensor.matmul(ph[:], w["w0"][:, i, H:], rb_[:],
                                     start=False, stop=True)
                    h = smallp.tile([H, 512], dt.bfloat16, tag="h", name="h")
                    nc.scalar.activation(h[:], ph[:], mybir.ActivationFunctionType.Relu,
                                         bias=w["b0"][:, i:i + 1], scale=1.0)
                    po = psump.tile([H, 512], dt.float32, tag="po", name="po")
                    nc.tensor.matmul(po[:], w["w1"][:, i, :], h[:],
                                     start=True, stop=False)
                    nc.tensor.matmul(po[:], w["ws"][:, i, :H], xa[:, sl],
                                     start=False, stop=False)
                    nc.tensor.matmul(po[:], w["ws"][:, i, H:], xb[:, sl],
                                     start=False, stop=True)
                    nc.scalar.activation(xa[:, sl], po[:],
                                         mybir.ActivationFunctionType.Copy)
                return xa

            def mean_stage(s, npm):
                for pl in range(NPLANES):
                    n1 = N1P[pl]
                    for c0 in range(0, n1, CHUNK):
                        wch = min(CHUNK, n1 - c0)
                        s1f = srp.tile([128, 1, CHUNK], dt.bfloat16, tag="sr", name="s1f")
                        sbuf_gather(s1f[:, :, :wch], npm[:],
                                    g1_t[pl][:, c0 // 16:(c0 + wch) // 16], wch)
                        acc = pmp.tile([H, CHUNK], dt.float32, tag="pm", name="acc")
                        nc.vector.tensor_copy(acc[:, :wch], s1f[:, 0, :wch])
                        for j in range(RMAX[pl] - 1):
                            w = min(CR[pl][j], c0 + wch) - c0
                            if w <= 0:
                                continue
                            srf = srp.tile([128, 1, CHUNK], dt.bfloat16,
                                           tag="sr", name="srf")
                            sbuf_gather(srf[:, :, :w], npm[:],
                                        gsum_t[pl][j][:, c0 // 16:(c0 + w) // 16], w)
                            srf32 = gp.tile([H, CHUNK], dt.float32, tag="g", name="srf32")
                            nc.vector.tensor_copy(srf32[:, :w], srf[:, 0, :w])
                            nc.vector.tensor_tensor(out=acc[:, :w], in0=acc[:, :w],
                                                    in1=srf32[:, :w],
                                                    op=mybir.AluOpType.add)
                        accb = srp.tile([128, 1, CHUNK], dt.bfloat16,
                                        tag="sr", name="accb")
                        nc.vector.tensor_copy(accb[:, 0, :wch], acc[:, :wch])
                        for sc0 in range(0, wch, SCHUNK):
                            wsc = min(SCHUNK, wch - sc0)
                            sums = srp.tile([128, SCHUNK // 128, C], dt.float32,
                                            tag="sums", name="sums", bufs=1)
                            for ch4 in range((wsc // 128 + 3) // 4):
                                pb = psump.tile([128, 512], dt.float32,
                                                tag="ph", name="pb")
                                nch = min(4, wsc // 128 - ch4 * 4)
                                for k in range(nch):
                                    chunk = ch4 * 4 + k
                                    nc.tensor.matmul(
                                        pb[:, k * C:(k + 1) * C],
                                        accb[:, 0, sc0 + chunk * 128:
                                             sc0 + (chunk + 1) * 128],
                                        W[s]["fcw"][:], start=True, stop=True)
                                nc.vector.tensor_copy(
                                    sums[:, ch4 * 4:ch4 * 4 + nch, :].rearrange(
                                        "p a f -> p (a f)"),
                                    pb[:, :nch * C])
                            nc.gpsimd.dma_scatter_add(
                                out_d[(s, pl)][:], sums[:, :wsc // 128, :],
                                sbin_t[pl][:, (c0 + sc0) // 16:(c0 + sc0 + wsc) // 16],
                                wsc, wsc, C, single_packet=False)

            # ---------------- schedule ----------------
            net = {}
            for rep in range(REPS):
                pT_t = npmp.tile([3, T], dt.bfloat16, tag="npm", name="pT_t")
                p2T_t = npmp.tile([3, T], dt.bfloat16, tag="npm", name="p2T_t")
                nc.sync.dma_start(pT_t[:], pT_d[:])
                nc.sync.dma_start(p2T_t[:], p2T_d[:])
                x0 = {"g": [actp.tile([H, T], dt.bfloat16, tag="netg", name="x0g0"),
                            pooledp.tile([H, T], dt.bfloat16, tag="pooled", name="x0g1")],
                      "c": [actp.tile([H, T], dt.bfloat16, tag="netc", name="x0c0"),
                            pooledp.tile([H, T], dt.bfloat16, tag="pooled", name="x0c1")]}
                for m in range(2):
                    for nt in range(T // 512):
                        sl = slice(nt * 512, (nt + 1) * 512)
                        ps_g = psump.tile([H, 512], dt.float32, tag="ph", name="ps_g")
                        ps_c = psump.tile([H, 512], dt.float32, tag="po", name="ps_c")
                        nc.tensor.matmul(ps_g[:], wp_t[:, m * H:(m + 1) * H],
                                         pT_t[:, sl], start=True, stop=True)
                        nc.tensor.matmul(ps_c[:], wp2_t[:, m * H:(m + 1) * H],
                                         p2T_t[:, sl], start=True, stop=True)
                        nc.scalar.activation(x0["g"][m][:, sl], ps_g[:],
                                             mybir.ActivationFunctionType.Copy)
                        nc.vector.tensor_tensor(out=x0["c"][m][:, sl],
                                                in0=x0["g"][m][:, sl],
                                                in1=ps_c[:], op=mybir.AluOpType.add)

                for s in ("g", "c"):
                    net[s] = resblock(s, 0, x0[s][0], x0[s][1])
                for i in range(1, NB):
                    npm = {}
                    for s in ("g", "c"):
                        npm[s] = make_net_pm(s, net[s])
                    pooled = {}
                    for s in ("g", "c"):
                        pooled[s] = pool_local(s, npm[s])
                    for s in ("g", "c"):
                        net[s] = resblock(s, i, net[s], pooled[s])
                for s in ("g", "c"):
                    npm_f = make_net_pm(s, net[s])
                    mean_stage(s, npm_f)

            if timing:
                chk_t = constp.tile([128, 128], dt.bfloat16)
                nc.vector.tensor_copy(chk_t[:], net["g"][:, :128])
                nc.sync.dma_start(chk_d[:], chk_t[:])

    nc.compile()

    in_maps = []
    for b in range(B):
        im = {
            "pT": np.ascontiguousarray(p[b].T).astype(BF),
            "p2T": np.ascontiguousarray(p2[b].T).astype(BF),
            "wp": wp.astype(BF), "wp2": wp2.astype(BF),
        }
        for s in ("g", "c"):
            sh = sh_host[s]
            w0pk = np.concatenate([sh["w0"][:, :H].transpose(1, 0, 2),
                                   sh["w0"][:, H:].transpose(1, 0, 2)], axis=2)
            wspk = np.concatenate([sh["ws"][:, :H].transpose(1, 0, 2),
                                   sh["ws"][:, H:].transpose(1, 0, 2)], axis=2)
            w1pk = sh["w1"].transpose(1, 0, 2)
            rb = np.zeros((H, NB, 2), F32)
            for i, (ba, bb) in enumerate(sh["relu_bias"]):
                rb[:, i, 0] = ba
                rb[:, i, 1] = bb
            im[f"{s}_w0"] = np.ascontiguousarray(w0pk).astype(BF)
            im[f"{s}_w1"] = np.ascontiguousarray(w1pk).astype(BF)
            im[f"{s}_ws"] = np.ascontiguousarray(wspk).astype(BF)
            im[f"{s}_rb"] = rb
            im[f"{s}_b0"] = np.ascontiguousarray(sh["b0"].T).astype(F32)
            im[f"{s}_fcw"] = fc_w[s].astype(BF)
        for pl in range(NPLANES):
            pr = preps[b][pl]
            im[f"g1_{pl}"] = wrap_idxs(pr.round_ids(1, N1P[pl], sum_pad=True))
            for j, r in enumerate(range(2, RMAX[pl] + 1)):
                im[f"gmax_{pl}_{r}"] = wrap_idxs(pr.round_ids(r, CR[pl][j], sum_pad=False))
                im[f"gsum_{pl}_{r}"] = wrap_idxs(pr.round_ids(r, CR[pl][j], sum_pad=True))
            im[f"pidx_{pl}"] = wrap_idxs(pr.pidx)
            empty = np.where(pr.cnt == 0)[0]
            sb = np.full(N1P[pl], int(empty[0]) if len(empty) else 0, np.int64)
            sb[:pr.n_occ] = pr.bins_sorted
            im[f"sbin_{pl}"] = wrap_idxs(sb)
        in_maps.append(im)

    return nc, in_maps, cvec


def _prep(inputs):
    p = np.asarray(inputs["p"], F32)
    idx_lists = compute_idx_lists(p)
    return [[PlanePrep(idx_lists[pl][b]) for pl in range(NPLANES)] for b in range(B)]


def kernel(**inputs):
    from concourse.bass_utils import run_bass_kernel_spmd

    preps = _prep(inputs)
    nc, in_maps, cvec = _build(inputs, preps, REPS=1, timing=False)
    res = run_bass_kernel_spmd(nc, in_maps, core_ids=list(range(B)))

    out = np.zeros((2 * NPLANES, B, C, R, R), F32)
    for b in range(B):
        for si, s in enumerate(("g", "c")):
            for pl in range(NPLANES):
                grid = np.asarray(res.results[b][f"out_{s}{pl}"], F32)
                pr = preps[b][pl]
                cnt = pr.cnt.astype(F32)
                true_sums = grid + cnt[:, None] * cvec[s][None, :]
                mean = true_sums / np.clip(cnt, 1.0, None)[:, None]
                mean[cnt == 0] = 0.0
                out[si * NPLANES + pl, b] = mean.T.reshape(C, R, R)
    return out


def measure_hw_time(inputs, reps=8, n_timing_runs=6):
    """Estimate per-iteration device time via in-kernel repetition differencing."""
    import time
    from concourse.bass_utils import run_bass_kernel_spmd

    preps = _prep(inputs)

    def runner(R_):
        nc, in_maps, _ = _build(inputs, preps, REPS=R_, timing=True)

        def once():
            t0 = time.perf_counter()
            run_bass_kernel_spmd(nc, in_maps, core_ids=list(range(B)))
            return time.perf_counter() - t0
        once()  # warm
        return min(once() for _ in range(n_timing_runs))

    t1 = runner(1)
    tR = runner(reps)
    per_iter = (tR - t1) / (reps - 1)
    return int(per_iter * 1e9), t1, tR


if __name__ == "__main__":
    import reference
    inputs = {k: np.asarray(v) for k, v in reference.setup_inputs().items()}
    result = kernel(**inputs)
    print("kernel output shape:", result.shape)



# revision 2
# speedup vs baseline: 58.2834x; 58.2834x over previous
"""TRN2 Bass kernel for nn_LocalPoolPointnetPPFusion (batch-parallel, 8 cores).

v2: pooling via gpsimd ap_gather (SBUF->SBUF access-pattern gather, ~0.4us/op
on HW) instead of SWDGE dma_gather (~7.6ns/token Q7 descriptor loop). All
activations stay feature-major; the two streams (geometry g / articulation c)
are interleaved per token as [128, T, 2] bf16 so one gather serves both.

Per-core pipeline:
  net_i[:, t, s] = (p @ wp)[t] (+ p2 @ wp2 for s=c)       (biases folded on host)
  5 resblocks per stream (in-place, bf16 matmuls, fp32 PSUM, strided token
  access into net_i); between blocks, per plane:
    strip = ap_gather(net_i, merged-round indices)   # one gather per plane
    per-bin max via ~R_max DVE tensor_tensor maxes on strip segments
    pooled += ap_gather(strip[:, :N1, :], pidx)      # expand back to tokens
  final stage: same strips with zero-padded indices, fp32 round sums ->
    per-bin sums -> @ fc_w on PE -> dma_scatter_add into [R*R, 2C] HBM grids.
  host folds deferred biases + fc bias + 1/cnt + transposes to [C, R, R].
"""
import sys
sys.path.insert(0, "/opt/trn_rl_repo")

import numpy as np
import ml_dtypes

BF = ml_dtypes.bfloat16
F32 = np.float32

B, T, H, C, R = 8, 8192, 128, 128, 128
NB = 5
NPLANES = 3
PLANE_COLS = ((0, 2), (0, 1), (1, 2))
TZ = T            # zero-token column in net_i
TP = T + 16       # net_i token-axis width (16 zero columns at the end)


def compute_idx_lists(p_np):
    import jax
    import jax.numpy as jnp
    cpu = jax.devices("cpu")[0]
    out = []
    with jax.default_device(cpu):
        pj = jnp.asarray(p_np)
        for cols in PLANE_COLS:
            xy = pj[..., jnp.array(cols)] / (1.0 + 0.0 + 1e-3) + 0.5
            xy = jnp.clip(xy, 0.0, 1.0 - 1e-3)
            g = jnp.floor(xy * R).astype(jnp.int32)
            out.append(np.asarray(g[..., 0] + R * g[..., 1]))
    return out


def wrap_idxs(flat):
    """token i -> idxs[i%16, i//16]; replicated to 128 partitions."""
    flat = np.asarray(flat, np.int64)
    n = len(flat)
    assert n % 16 == 0
    a = flat.reshape(n // 16, 16).T.astype(np.int16)
    return np.tile(a, (8, 1))


def align(x, a):
    return (int(x) + a - 1) // a * a


class PlanePrep:
    def __init__(self, idx):
        self.idx = idx
        cnt = np.bincount(idx, minlength=R * R)
        self.cnt = cnt
        occ = np.where(cnt > 0)[0]
        order = np.argsort(-cnt[occ], kind="stable")
        self.bins_sorted = occ[order]
        self.n_occ = len(occ)
        self.occ_sorted = cnt[self.bins_sorted]
        sort_by_bin = np.argsort(idx, kind="stable")
        starts = np.searchsorted(idx[sort_by_bin], self.bins_sorted)
        self.members = [sort_by_bin[s:s + k] for s, k in zip(starts, self.occ_sorted)]
        slot_of_bin = np.full(R * R, -1, np.int64)
        slot_of_bin[self.bins_sorted] = np.arange(self.n_occ)
        self.pidx = slot_of_bin[idx]
        self.R_max = int(self.occ_sorted[0])
        self.n_r = [int((self.occ_sorted >= r).sum()) for r in range(1, self.R_max + 1)]

    def nr(self, r):
        return self.n_r[r - 1] if r <= self.R_max else 0

    def round_ids(self, r, width, zero_pad):
        """Indices for round r, padded to `width`. zero_pad=True pads with the
        zero token TZ (for sums); False pads with the bin's first member
        (self-max no-op) or token of slot 0 for slots beyond n_occ."""
        ids = np.full(width, TZ if zero_pad else int(self.members[0][0]), np.int64)
        nr = self.nr(r)
        for s in range(min(nr, width)):
            ids[s] = self.members[s][r - 1]
        if not zero_pad:
            for s in range(nr, width):
                if s < self.n_occ:
                    ids[s] = self.members[s][0]
        return ids


def _prep(inputs):
    p = np.asarray(inputs["p"], F32)
    idx_lists = compute_idx_lists(p)
    return [[PlanePrep(idx_lists[pl][b]) for pl in range(NPLANES)] for b in range(B)]


def _build(inputs, preps, REPS=1, timing=False):
    import concourse.bacc as bacc
    import concourse.tile as tile
    from concourse import mybir

    p = np.asarray(inputs["p"], F32)
    p2 = np.asarray(inputs["p2"], F32)

    # ---- shared (cross-batch) strip geometry per plane ----
    RMAX = [max(preps[b][pl].R_max for b in range(B)) for pl in range(NPLANES)]
    WR, OFF, N1, W = [], [], [], []
    for pl in range(NPLANES):
        wr = [align(max(preps[b][pl].n_occ for b in range(B)), 128)]
        for r in range(2, RMAX[pl] + 1):
            wr.append(align(max(preps[b][pl].nr(r) for b in range(B)), 16))
        off = np.concatenate([[0], np.cumsum(wr)])
        WR.append(wr)
        OFF.append(off)
        N1.append(wr[0])
        W.append(int(off[-1]))

    # ---- host-side weight/bias folding (identical to v1) ----
    def stream_host(pref, base_bias):
        w0 = np.asarray(inputs[f"{pref}_w0"], F32)
        b0 = np.asarray(inputs[f"{pref}_b0"], F32)
        w1 = np.asarray(inputs[f"{pref}_w1"], F32)
        b1 = np.asarray(inputs[f"{pref}_b1"], F32)
        ws = np.asarray(inputs[f"{pref}_ws"], F32)
        relu_bias = []
        Bp = base_bias
        for i in range(NB):
            if i == 0:
                bias_in = Bp
                relu_bias.append((bias_in[:H].copy(), bias_in[H:].copy()))
            else:
                bias_in = np.concatenate([Bp, 3.0 * Bp])
                relu_bias.append((Bp.copy(), 3.0 * Bp))
            Bp = b1[i] + bias_in @ ws[i]
        return dict(w0=w0, b0=b0, w1=w1, ws=ws, relu_bias=relu_bias, B_final=Bp)

    wp = np.asarray(inputs["wp"], F32)
    bp = np.asarray(inputs["bp"], F32)
    wp2 = np.asarray(inputs["wp2"], F32)
    bp2 = np.asarray(inputs["bp2"], F32)
    sh_host = {"g": stream_host("blk", bp.copy()), "c": stream_host("blkc", bp + bp2)}
    fc_w = {"g": np.asarray(inputs["fc_c_w"], F32),
            "c": np.asarray(inputs["fc_cc_w"], F32)}
    fc_b = {"g": np.asarray(inputs["fc_c_b"], F32),
            "c": np.asarray(inputs["fc_cc_b"], F32)}
    cvec = {s: sh_host[s]["B_final"] @ fc_w[s] + fc_b[s] for s in ("g", "c")}

    nc = bacc.Bacc("TRN2", target_bir_lowering=False, debug=False, num_devices=B)
    dt = mybir.dt

    def din(name, shape, dtype):
        return nc.dram_tensor(name, shape, dtype, kind="ExternalInput")

    pT_d = din("pT", [3, T], dt.bfloat16)
    p2T_d = din("p2T", [3, T], dt.bfloat16)
    wp_d = din("wp", [3, 2 * H], dt.bfloat16)
    wp2_d = din("wp2", [3, 2 * H], dt.bfloat16)
    wpk_d = {}
    for s in ("g", "c"):
        wpk_d[s] = dict(
            w0=din(f"{s}_w0", [H, NB, 2 * H], dt.bfloat16),
            w1=din(f"{s}_w1", [H, NB, H], dt.bfloat16),
            ws=din(f"{s}_ws", [H, NB, 2 * H], dt.bfloat16),
            rb=din(f"{s}_rb", [H, NB, 2], dt.float32),
            b0=din(f"{s}_b0", [H, NB], dt.float32),
            fcw=din(f"{s}_fcw", [H, C], dt.bfloat16),
        )
    smax_d = [din(f"smax_{pl}", [128, W[pl] // 16], dt.int16) for pl in range(NPLANES)]
    ssum_d = [din(f"ssum_{pl}", [128, W[pl] // 16], dt.int16) for pl in range(NPLANES)]
    pidx_d = [din(f"pidx_{pl}", [128, T // 16], dt.int16) for pl in range(NPLANES)]
    sbin_d = [din(f"sbin_{pl}", [128, N1[pl] // 16], dt.int16) for pl in range(NPLANES)]

    out_kind = "Internal" if timing else "ExternalOutput"
    out_d = {pl: nc.dram_tensor(f"out_{pl}", [R * R, 2 * C], dt.float32, kind=out_kind)
             for pl in range(NPLANES)}
    chk_d = nc.dram_tensor("chk", [128, 128], dt.bfloat16, kind="ExternalOutput") \
        if timing else None

    SI = {"g": 0, "c": 1}

    with tile.TileContext(nc) as tc:
        with tc.tile_pool(name="const", bufs=1) as constp, \
             tc.tile_pool(name="net", bufs=1) as netp, \
             tc.tile_pool(name="pool", bufs=1) as poolp, \
             tc.tile_pool(name="strip", bufs=2) as stripp, \
             tc.tile_pool(name="small", bufs=2) as smallp, \
             tc.tile_pool(name="psum", bufs=2, space="PSUM") as psump:

            wp_t = constp.tile([3, 2 * H], dt.bfloat16)
            wp2_t = constp.tile([3, 2 * H], dt.bfloat16)
            nc.sync.dma_start(wp_t[:], wp_d[:])
            nc.sync.dma_start(wp2_t[:], wp2_d[:])
            Wt = {}
            for s in ("g", "c"):
                Wt[s] = dict(
                    w0=constp.tile([H, NB, 2 * H], dt.bfloat16, tag=f"{s}w0", name=f"{s}w0"),
                    w1=constp.tile([H, NB, H], dt.bfloat16, tag=f"{s}w1", name=f"{s}w1"),
                    ws=constp.tile([H, NB, 2 * H], dt.bfloat16, tag=f"{s}ws", name=f"{s}ws"),
                    rb=constp.tile([H, NB, 2], dt.float32, tag=f"{s}rb", name=f"{s}rb"),
                    b0=constp.tile([H, NB], dt.float32, tag=f"{s}b0", name=f"{s}b0"),
                    fcw=constp.tile([H, C], dt.bfloat16, tag=f"{s}fcw", name=f"{s}fcw"),
                )
                for k, t in Wt[s].items():
                    nc.sync.dma_start(t[:], wpk_d[s][k][:])
            smax_t, ssum_t, pidx_t, sbin_t = [], [], [], []
            for pl in range(NPLANES):
                smax_t.append(constp.tile([128, W[pl] // 16], dt.int16,
                                          tag=f"sm{pl}", name=f"smt{pl}"))
                ssum_t.append(constp.tile([128, W[pl] // 16], dt.int16,
                                          tag=f"ss{pl}", name=f"sst{pl}"))
                pidx_t.append(constp.tile([128, T // 16], dt.int16,
                                          tag=f"pi{pl}", name=f"pit{pl}"))
                sbin_t.append(constp.tile([128, N1[pl] // 16], dt.int16,
                                          tag=f"sb{pl}", name=f"sbt{pl}"))
                nc.sync.dma_start(smax_t[pl][:], smax_d[pl][:])
                nc.sync.dma_start(ssum_t[pl][:], ssum_d[pl][:])
                nc.sync.dma_start(pidx_t[pl][:], pidx_d[pl][:])
                nc.sync.dma_start(sbin_t[pl][:], sbin_d[pl][:])

            def sview(t, sl, s):
                """[H, len(sl)] strided per-stream view of an interleaved tile."""
                si = SI[s]
                return t[:, sl, si:si + 1].rearrange("p t s -> p (t s)")

            def resblock(s, i, xa_t, xb_t):
                """xa/xb are interleaved tiles; stream s slices. In-place on xa."""
                w = Wt[s]
                ba_ap = w["rb"][:, i, 0:1]
                bb_ap = w["rb"][:, i, 1:2]
                for nt in range(T // 512):
                    sl = slice(nt * 512, (nt + 1) * 512)
                    xa = sview(xa_t, sl, s)
                    xb = sview(xb_t, sl, s)
                    ra = smallp.tile([H, 512], dt.bfloat16, tag="ra", name="ra")
                    rb_ = smallp.tile([H, 512], dt.bfloat16, tag="rb", name="rb")
                    nc.vector.tensor_scalar(out=ra[:], in0=xa, scalar1=ba_ap,
                                            scalar2=0.0, op0=mybir.AluOpType.add,
                                            op1=mybir.AluOpType.max)
                    nc.vector.tensor_scalar(out=rb_[:], in0=xb, scalar1=bb_ap,
                                            scalar2=0.0, op0=mybir.AluOpType.add,
                                            op1=mybir.AluOpType.max)
                    ph = psump.tile([H, 512], dt.float32, tag="ph", name="ph")
                    nc.tensor.matmul(ph[:], w["w0"][:, i, :H], ra[:],
                                     start=True, stop=False)
                    nc.tensor.matmul(ph[:], w["w0"][:, i, H:], rb_[:],
                                     start=False, stop=True)
                    h = smallp.tile([H, 512], dt.bfloat16, tag="h", name="h")
                    nc.scalar.activation(h[:], ph[:], mybir.ActivationFunctionType.Relu,
                                         bias=w["b0"][:, i:i + 1], scale=1.0)
                    po = psump.tile([H, 512], dt.float32, tag="po", name="po")
                    nc.tensor.matmul(po[:], w["w1"][:, i, :], h[:],
                                     start=True, stop=False)
                    nc.tensor.matmul(po[:], w["ws"][:, i, :H], xa,
                                     start=False, stop=False)
                    nc.tensor.matmul(po[:], w["ws"][:, i, H:], xb,
                                     start=False, stop=True)
                    nc.scalar.activation(xa, po[:],
                                         mybir.ActivationFunctionType.Copy)

            # ---------------- schedule ----------------
            for rep in range(REPS):
                net_i = netp.tile([128, TP, 2], dt.bfloat16, tag="net", name="net_i")
                pooled_i = poolp.tile([128, T, 2], dt.bfloat16, tag="pool",
                                      name="pooled_i")
                nc.vector.memset(net_i[:, T:TP, :].rearrange("p t s -> p (t s)"), 0.0)

                pT_t = stripp.tile([3, T], dt.bfloat16, tag="strip", name="pT_t")
                p2T_t = stripp.tile([3, T], dt.bfloat16, tag="strip", name="p2T_t")
                nc.sync.dma_start(pT_t[:], pT_d[:])
                nc.sync.dma_start(p2T_t[:], p2T_d[:])

                for m, dst in ((0, net_i), (1, pooled_i)):
                    for nt in range(T // 512):
                        sl = slice(nt * 512, (nt + 1) * 512)
                        ps_g = psump.tile([H, 512], dt.float32, tag="ph", name="ps_g")
                        ps_c = psump.tile([H, 512], dt.float32, tag="po", name="ps_c")
                        nc.tensor.matmul(ps_g[:], wp_t[:, m * H:(m + 1) * H],
                                         pT_t[:, sl], start=True, stop=True)
                        nc.tensor.matmul(ps_c[:], wp2_t[:, m * H:(m + 1) * H],
                                         p2T_t[:, sl], start=True, stop=True)
                        dg = sview(dst, sl, "g")
                        dc = sview(dst, sl, "c")
                        nc.scalar.activation(dg, ps_g[:],
                                             mybir.ActivationFunctionType.Copy)
                        nc.vector.tensor_tensor(out=dc, in0=dg, in1=ps_c[:],
                                                op=mybir.AluOpType.add)

                for s in ("g", "c"):
                    resblock(s, 0, net_i, pooled_i)

                for i in range(1, NB):
                    for pl in range(NPLANES):
                        strip = stripp.tile([128, W[pl], 2], dt.bfloat16,
                                            tag="strip", name=f"strip{pl}")
                        nc.gpsimd.ap_gather(strip[:], net_i[:], smax_t[pl][:],
                                            128, TP, 2, W[pl])
                        for j, r in enumerate(range(2, RMAX[pl] + 1)):
                            wr = WR[pl][j + 1]
                            o = int(OFF[pl][j + 1])
                            nc.vector.tensor_tensor(
                                out=strip[:, :wr, :], in0=strip[:, :wr, :],
                                in1=strip[:, o:o + wr, :], op=mybir.AluOpType.max)
                        if pl == 0:
                            nc.gpsimd.ap_gather(pooled_i[:], strip[:, :N1[pl], :],
                                                pidx_t[pl][:], 128, N1[pl], 2, T)
                        else:
                            exp = stripp.tile([128, T, 2], dt.bfloat16,
                                              tag="strip", name=f"exp{pl}")
                            nc.gpsimd.ap_gather(exp[:], strip[:, :N1[pl], :],
                                                pidx_t[pl][:], 128, N1[pl], 2, T)
                            nc.vector.tensor_tensor(out=pooled_i[:], in0=pooled_i[:],
                                                    in1=exp[:], op=mybir.AluOpType.add)
                    for s in ("g", "c"):
                        resblock(s, i, net_i, pooled_i)

                # ---- mean stage ----
                for pl in range(NPLANES):
                    strip = stripp.tile([128, W[pl], 2], dt.bfloat16,
                                        tag="strip", name=f"mstrip{pl}")
                    nc.gpsimd.ap_gather(strip[:], net_i[:], ssum_t[pl][:],
                                        128, TP, 2, W[pl])
                    acc = poolp.tile([128, N1[pl], 2], dt.float32, tag="pool",
                                     name=f"acc{pl}")
                    nc.vector.tensor_copy(acc[:], strip[:, :N1[pl], :])
                    for j, r in enumerate(range(2, RMAX[pl] + 1)):
                        wr = WR[pl][j + 1]
                        o = int(OFF[pl][j + 1])
                        nc.vector.tensor_tensor(
                            out=acc[:, :wr, :], in0=acc[:, :wr, :],
                            in1=strip[:, o:o + wr, :], op=mybir.AluOpType.add)
                    accb = stripp.tile([128, 2, N1[pl]], dt.bfloat16, tag="strip",
                                       name=f"accb{pl}")
                    for s in ("g", "c"):
                        si = SI[s]
                        nc.vector.tensor_copy(
                            accb[:, si, :],
                            acc[:, :, si:si + 1].rearrange("p t s -> p (t s)"))
                    nch = N1[pl] // 128
                    sums = stripp.tile([128, nch, 2 * C], dt.float32, tag="strip",
                                       name=f"sums{pl}")
                    for k in range(nch):
                        pb = psump.tile([128, 2 * C], dt.float32, tag="ph", name="pb")
                        ksl = slice(k * 128, (k + 1) * 128)
                        nc.tensor.matmul(pb[:, :C], accb[:, 0, ksl], Wt["g"]["fcw"][:],
                                         start=True, stop=True)
                        nc.tensor.matmul(pb[:, C:], accb[:, 1, ksl], Wt["c"]["fcw"][:],
                                         start=True, stop=True)
                        nc.vector.tensor_copy(sums[:, k, :], pb[:])
                    nc.gpsimd.dma_scatter_add(
                        out_d[pl][:], sums[:], sbin_t[pl][:],
                        N1[pl], N1[pl], 2 * C, single_packet=False)

            if timing:
                chk_t = constp.tile([128, 128], dt.bfloat16)
                nc.vector.tensor_copy(chk_t[:], net_i[:, :64, :].rearrange(
                    "p t s -> p (t s)"))
                nc.sync.dma_start(chk_d[:], chk_t[:])

    nc.compile()

    # ---- per-core input maps ----
    in_maps = []
    for b in range(B):
        im = {
            "pT": np.ascontiguousarray(p[b].T).astype(BF),
            "p2T": np.ascontiguousarray(p2[b].T).astype(BF),
            "wp": wp.astype(BF), "wp2": wp2.astype(BF),
        }
        for s in ("g", "c"):
            sh = sh_host[s]
            w0pk = np.concatenate([sh["w0"][:, :H].transpose(1, 0, 2),
                                   sh["w0"][:, H:].transpose(1, 0, 2)], axis=2)
            wspk = np.concatenate([sh["ws"][:, :H].transpose(1, 0, 2),
                                   sh["ws"][:, H:].transpose(1, 0, 2)], axis=2)
            w1pk = sh["w1"].transpose(1, 0, 2)
            rb = np.zeros((H, NB, 2), F32)
            for i, (ba, bb) in enumerate(sh["relu_bias"]):
                rb[:, i, 0] = ba
                rb[:, i, 1] = bb
            im[f"{s}_w0"] = np.ascontiguousarray(w0pk).astype(BF)
            im[f"{s}_w1"] = np.ascontiguousarray(w1pk).astype(BF)
            im[f"{s}_ws"] = np.ascontiguousarray(wspk).astype(BF)
            im[f"{s}_rb"] = rb
            im[f"{s}_b0"] = np.ascontiguousarray(sh["b0"].T).astype(F32)
            im[f"{s}_fcw"] = fc_w[s].astype(BF)
        for pl in range(NPLANES):
            pr = preps[b][pl]
            smax = np.concatenate(
                [pr.round_ids(r, WR[pl][r - 1], zero_pad=False)
                 for r in range(1, RMAX[pl] + 1)])
            ssum = np.concatenate(
                [pr.round_ids(r, WR[pl][r - 1], zero_pad=True)
                 for r in range(1, RMAX[pl] + 1)])
            im[f"smax_{pl}"] = wrap_idxs(smax)
            im[f"ssum_{pl}"] = wrap_idxs(ssum)
            im[f"pidx_{pl}"] = wrap_idxs(pr.pidx)
            empty = np.where(pr.cnt == 0)[0]
            sb = np.full(N1[pl], int(empty[0]) if len(empty) else 0, np.int64)
            sb[:pr.n_occ] = pr.bins_sorted
            im[f"sbin_{pl}"] = wrap_idxs(sb)
        in_maps.append(im)

    return nc, in_maps, cvec


def kernel(**inputs):
    from concourse.bass_utils import run_bass_kernel_spmd

    preps = _prep(inputs)
    nc, in_maps, cvec = _build(inputs, preps, REPS=1, timing=False)
    res = run_bass_kernel_spmd(nc, in_maps, core_ids=list(range(B)))

    out = np.zeros((2 * NPLANES, B, C, R, R), F32)
    for b in range(B):
        for pl in range(NPLANES):
            grid = np.asarray(res.results[b][f"out_{pl}"], F32)
            pr = preps[b][pl]
            cnt = pr.cnt.astype(F32)
            for si, s in enumerate(("g", "c")):
                part = grid[:, si * C:(si + 1) * C]
                true_sums = part + cnt[:, None] * cvec[s][None, :]
                mean = true_sums / np.clip(cnt, 1.0, None)[:, None]
                mean[cnt == 0] = 0.0
                out[si * NPLANES + pl, b] = mean.T.reshape(C, R, R)
    return out


if __name__ == "__main__":
    import reference
    inputs = {k: np.asarray(v) for k, v in reference.setup_inputs().items()}
    result = kernel(**inputs)
    print("kernel output shape:", result.shape)


# revision 5
# speedup vs baseline: 59.7604x; 1.0253x over previous
"""TRN2 Bass kernel for nn_LocalPoolPointnetPPFusion (batch-parallel, 8 cores).

v2: pooling via gpsimd ap_gather (SBUF->SBUF access-pattern gather, ~0.4us/op
on HW) instead of SWDGE dma_gather (~7.6ns/token Q7 descriptor loop). All
activations stay feature-major; the two streams (geometry g / articulation c)
are interleaved per token as [128, T, 2] bf16 so one gather serves both.

Per-core pipeline:
  net_i[:, t, s] = (p @ wp)[t] (+ p2 @ wp2 for s=c)       (biases folded on host)
  5 resblocks per stream (in-place, bf16 matmuls, fp32 PSUM, strided token
  access into net_i); between blocks, per plane:
    strip = ap_gather(net_i, merged-round indices)   # one gather per plane
    per-bin max via ~R_max DVE tensor_tensor maxes on strip segments
    pooled += ap_gather(strip[:, :N1, :], pidx)      # expand back to tokens
  final stage: same strips with zero-padded indices, fp32 round sums ->
    per-bin sums -> @ fc_w on PE -> dma_scatter_add into [R*R, 2C] HBM grids.
  host folds deferred biases + fc bias + 1/cnt + transposes to [C, R, R].
"""
import sys
sys.path.insert(0, "/opt/trn_rl_repo")

import numpy as np
import ml_dtypes

BF = ml_dtypes.bfloat16
F32 = np.float32

B, T, H, C, R = 8, 8192, 128, 128, 128
NB = 5
NPLANES = 3
PLANE_COLS = ((0, 2), (0, 1), (1, 2))
TZ = T            # zero-token column in net_i
TP = T + 16       # net_i token-axis width (16 zero columns at the end)


def compute_idx_lists(p_np):
    import jax
    import jax.numpy as jnp
    cpu = jax.devices("cpu")[0]
    out = []
    with jax.default_device(cpu):
        pj = jnp.asarray(p_np)
        for cols in PLANE_COLS:
            xy = pj[..., jnp.array(cols)] / (1.0 + 0.0 + 1e-3) + 0.5
            xy = jnp.clip(xy, 0.0, 1.0 - 1e-3)
            g = jnp.floor(xy * R).astype(jnp.int32)
            out.append(np.asarray(g[..., 0] + R * g[..., 1]))
    return out


def wrap_idxs(flat):
    """token i -> idxs[i%16, i//16]; replicated to 128 partitions."""
    flat = np.asarray(flat, np.int64)
    n = len(flat)
    assert n % 16 == 0
    a = flat.reshape(n // 16, 16).T.astype(np.int16)
    return np.tile(a, (8, 1))


def align(x, a):
    return (int(x) + a - 1) // a * a


class PlanePrep:
    def __init__(self, idx):
        self.idx = idx
        cnt = np.bincount(idx, minlength=R * R)
        self.cnt = cnt
        occ = np.where(cnt > 0)[0]
        order = np.argsort(-cnt[occ], kind="stable")
        self.bins_sorted = occ[order]
        self.n_occ = len(occ)
        self.occ_sorted = cnt[self.bins_sorted]
        sort_by_bin = np.argsort(idx, kind="stable")
        starts = np.searchsorted(idx[sort_by_bin], self.bins_sorted)
        self.members = [sort_by_bin[s:s + k] for s, k in zip(starts, self.occ_sorted)]
        slot_of_bin = np.full(R * R, -1, np.int64)
        slot_of_bin[self.bins_sorted] = np.arange(self.n_occ)
        self.pidx = slot_of_bin[idx]
        self.R_max = int(self.occ_sorted[0])
        self.n_r = [int((self.occ_sorted >= r).sum()) for r in range(1, self.R_max + 1)]

    def nr(self, r):
        return self.n_r[r - 1] if r <= self.R_max else 0

    def round_ids(self, r, width, zero_pad):
        """Indices for round r, padded to `width`. zero_pad=True pads with the
        zero token TZ (for sums); False pads with the bin's first member
        (self-max no-op) or token of slot 0 for slots beyond n_occ."""
        ids = np.full(width, TZ if zero_pad else int(self.members[0][0]), np.int64)
        nr = self.nr(r)
        for s in range(min(nr, width)):
            ids[s] = self.members[s][r - 1]
        if not zero_pad:
            for s in range(nr, width):
                if s < self.n_occ:
                    ids[s] = self.members[s][0]
        return ids


def _prep(inputs):
    p = np.asarray(inputs["p"], F32)
    idx_lists = compute_idx_lists(p)
    return [[PlanePrep(idx_lists[pl][b]) for pl in range(NPLANES)] for b in range(B)]


def _build(inputs, preps, REPS=1, timing=False):
    import concourse.bacc as bacc
    import concourse.tile as tile
    from concourse import mybir

    p = np.asarray(inputs["p"], F32)
    p2 = np.asarray(inputs["p2"], F32)

    # ---- shared (cross-batch) strip geometry per plane ----
    RMAX = [max(preps[b][pl].R_max for b in range(B)) for pl in range(NPLANES)]
    WR, OFF, N1, W = [], [], [], []
    for pl in range(NPLANES):
        wr = [align(max(preps[b][pl].n_occ for b in range(B)), 128)]
        for r in range(2, RMAX[pl] + 1):
            wr.append(align(max(preps[b][pl].nr(r) for b in range(B)), 16))
        off = np.concatenate([[0], np.cumsum(wr)])
        WR.append(wr)
        OFF.append(off)
        N1.append(wr[0])
        W.append(int(off[-1]))

    # ---- host-side weight/bias folding (identical to v1) ----
    def stream_host(pref, base_bias):
        w0 = np.asarray(inputs[f"{pref}_w0"], F32)
        b0 = np.asarray(inputs[f"{pref}_b0"], F32)
        w1 = np.asarray(inputs[f"{pref}_w1"], F32)
        b1 = np.asarray(inputs[f"{pref}_b1"], F32)
        ws = np.asarray(inputs[f"{pref}_ws"], F32)
        relu_bias = []
        Bp = base_bias
        for i in range(NB):
            if i == 0:
                bias_in = Bp
                relu_bias.append((bias_in[:H].copy(), bias_in[H:].copy()))
            else:
                bias_in = np.concatenate([Bp, 3.0 * Bp])
                relu_bias.append((Bp.copy(), 3.0 * Bp))
            Bp = b1[i] + bias_in @ ws[i]
        return dict(w0=w0, b0=b0, w1=w1, ws=ws, relu_bias=relu_bias, B_final=Bp)

    wp = np.asarray(inputs["wp"], F32)
    bp = np.asarray(inputs["bp"], F32)
    wp2 = np.asarray(inputs["wp2"], F32)
    bp2 = np.asarray(inputs["bp2"], F32)
    sh_host = {"g": stream_host("blk", bp.copy()), "c": stream_host("blkc", bp + bp2)}
    fc_w = {"g": np.asarray(inputs["fc_c_w"], F32),
            "c": np.asarray(inputs["fc_cc_w"], F32)}
    fc_b = {"g": np.asarray(inputs["fc_c_b"], F32),
            "c": np.asarray(inputs["fc_cc_b"], F32)}
    cvec = {s: sh_host[s]["B_final"] @ fc_w[s] + fc_b[s] for s in ("g", "c")}

    nc = bacc.Bacc("TRN2", target_bir_lowering=False, debug=False, num_devices=B)
    dt = mybir.dt

    def din(name, shape, dtype):
        return nc.dram_tensor(name, shape, dtype, kind="ExternalInput")

    pT_d = din("pT", [3, T], dt.bfloat16)
    p2T_d = din("p2T", [3, T], dt.bfloat16)
    wp_d = din("wp", [3, 2 * H], dt.bfloat16)
    wp2_d = din("wp2", [3, 2 * H], dt.bfloat16)
    wpk_d = {}
    for s in ("g", "c"):
        wpk_d[s] = dict(
            w0=din(f"{s}_w0", [H, NB, 2 * H], dt.bfloat16),
            w1=din(f"{s}_w1", [H, NB, H], dt.bfloat16),
            ws=din(f"{s}_ws", [H, NB, 2 * H], dt.bfloat16),
            rb=din(f"{s}_rb", [H, NB, 2], dt.float32),
            b0=din(f"{s}_b0", [H, NB], dt.float32),
            fcw=din(f"{s}_fcw", [H, C], dt.bfloat16),
        )
    smax_d = [din(f"smax_{pl}", [128, W[pl] // 16], dt.int16) for pl in range(NPLANES)]
    ssum_d = [din(f"ssum_{pl}", [128, W[pl] // 16], dt.int16) for pl in range(NPLANES)]
    pidx_d = [din(f"pidx_{pl}", [128, T // 16], dt.int16) for pl in range(NPLANES)]
    sbin_d = [din(f"sbin_{pl}", [128, N1[pl] // 16], dt.int16) for pl in range(NPLANES)]

    out_kind = "Internal" if timing else "ExternalOutput"
    out_d = {pl: nc.dram_tensor(f"out_{pl}", [R * R, 2 * C], dt.float32, kind=out_kind)
             for pl in range(NPLANES)}
    chk_d = nc.dram_tensor("chk", [128, 128], dt.bfloat16, kind="ExternalOutput") \
        if timing else None

    SI = {"g": 0, "c": 1}

    with tile.TileContext(nc) as tc:
        with tc.tile_pool(name="const", bufs=1) as constp, \
             tc.tile_pool(name="net", bufs=1) as netp, \
             tc.tile_pool(name="pool", bufs=1) as poolp, \
             tc.tile_pool(name="strip", bufs=3) as stripp, \
             tc.tile_pool(name="exp", bufs=1) as expp, \
             tc.tile_pool(name="small", bufs=2) as smallp, \
             tc.tile_pool(name="psum", bufs=2, space="PSUM") as psump:

            wp_t = constp.tile([3, 2 * H], dt.bfloat16)
            wp2_t = constp.tile([3, 2 * H], dt.bfloat16)
            nc.sync.dma_start(wp_t[:], wp_d[:])
            nc.sync.dma_start(wp2_t[:], wp2_d[:])
            Wt = {}
            for s in ("g", "c"):
                Wt[s] = dict(
                    w0=constp.tile([H, NB, 2 * H], dt.bfloat16, tag=f"{s}w0", name=f"{s}w0"),
                    w1=constp.tile([H, NB, H], dt.bfloat16, tag=f"{s}w1", name=f"{s}w1"),
                    ws=constp.tile([H, NB, 2 * H], dt.bfloat16, tag=f"{s}ws", name=f"{s}ws"),
                    rb=constp.tile([H, NB, 2], dt.float32, tag=f"{s}rb", name=f"{s}rb"),
                    b0=constp.tile([H, NB], dt.float32, tag=f"{s}b0", name=f"{s}b0"),
                    fcw=constp.tile([H, C], dt.bfloat16, tag=f"{s}fcw", name=f"{s}fcw"),
                )
                for k, t in Wt[s].items():
                    nc.sync.dma_start(t[:], wpk_d[s][k][:])
            smax_t, ssum_t, pidx_t, sbin_t = [], [], [], []
            for pl in range(NPLANES):
                smax_t.append(constp.tile([128, W[pl] // 16], dt.int16,
                                          tag=f"sm{pl}", name=f"smt{pl}"))
                ssum_t.append(constp.tile([128, W[pl] // 16], dt.int16,
                                          tag=f"ss{pl}", name=f"sst{pl}"))
                pidx_t.append(constp.tile([128, T // 16], dt.int16,
                                          tag=f"pi{pl}", name=f"pit{pl}"))
                sbin_t.append(constp.tile([128, N1[pl] // 16], dt.int16,
                                          tag=f"sb{pl}", name=f"sbt{pl}"))
                nc.sync.dma_start(smax_t[pl][:], smax_d[pl][:])
                nc.sync.dma_start(ssum_t[pl][:], ssum_d[pl][:])
                nc.sync.dma_start(pidx_t[pl][:], pidx_d[pl][:])
                nc.sync.dma_start(sbin_t[pl][:], sbin_d[pl][:])

            def sview(t, sl, s):
                """[H, len(sl)] strided per-stream view of an interleaved tile."""
                si = SI[s]
                return t[:, sl, si:si + 1].rearrange("p t s -> p (t s)")

            def resblock(s, i, xa_t, xb_t):
                """xa/xb are interleaved tiles; stream s slices. In-place on xa."""
                w = Wt[s]
                ba_ap = w["rb"][:, i, 0:1]
                bb_ap = w["rb"][:, i, 1:2]
                for nt in range(T // 512):
                    sl = slice(nt * 512, (nt + 1) * 512)
                    xa = sview(xa_t, sl, s)
                    xb = sview(xb_t, sl, s)
                    ra = smallp.tile([H, 512], dt.bfloat16, tag="ra", name="ra")
                    rb_ = smallp.tile([H, 512], dt.bfloat16, tag="rb", name="rb")
                    nc.vector.tensor_scalar(out=ra[:], in0=xa, scalar1=ba_ap,
                                            scalar2=0.0, op0=mybir.AluOpType.add,
                                            op1=mybir.AluOpType.max)
                    nc.vector.tensor_scalar(out=rb_[:], in0=xb, scalar1=bb_ap,
                                            scalar2=0.0, op0=mybir.AluOpType.add,
                                            op1=mybir.AluOpType.max)
                    ph = psump.tile([H, 512], dt.float32, tag="ph", name="ph")
                    nc.tensor.matmul(ph[:], w["w0"][:, i, :H], ra[:],
                                     start=True, stop=False)
                    nc.tensor.matmul(ph[:], w["w0"][:, i, H:], rb_[:],
                                     start=False, stop=True)
                    h = smallp.tile([H, 512], dt.bfloat16, tag="h", name="h")
                    nc.scalar.activation(h[:], ph[:], mybir.ActivationFunctionType.Relu,
                                         bias=w["b0"][:, i:i + 1], scale=1.0)
                    po = psump.tile([H, 512], dt.float32, tag="po", name="po")
                    nc.tensor.matmul(po[:], w["w1"][:, i, :], h[:],
                                     start=True, stop=False)
                    nc.tensor.matmul(po[:], w["ws"][:, i, :H], xa,
                                     start=False, stop=False)
                    nc.tensor.matmul(po[:], w["ws"][:, i, H:], xb,
                                     start=False, stop=True)
                    nc.scalar.activation(xa, po[:],
                                         mybir.ActivationFunctionType.Copy)

            # ---------------- schedule ----------------
            for rep in range(REPS):
                net_i = netp.tile([128, TP, 2], dt.bfloat16, tag="net", name="net_i")
                pooled_i = poolp.tile([128, T, 2], dt.bfloat16, tag="pool",
                                      name="pooled_i")
                nc.vector.memset(net_i[:, T:TP, :].rearrange("p t s -> p (t s)"), 0.0)

                pT_t = stripp.tile([3, T], dt.bfloat16, tag="strip", name="pT_t")
                p2T_t = stripp.tile([3, T], dt.bfloat16, tag="strip", name="p2T_t")
                nc.sync.dma_start(pT_t[:], pT_d[:])
                nc.sync.dma_start(p2T_t[:], p2T_d[:])

                for m, dst in ((0, net_i), (1, pooled_i)):
                    for nt in range(T // 512):
                        sl = slice(nt * 512, (nt + 1) * 512)
                        ps_g = psump.tile([H, 512], dt.float32, tag="ph", name="ps_g")
                        ps_c = psump.tile([H, 512], dt.float32, tag="po", name="ps_c")
                        nc.tensor.matmul(ps_g[:], wp_t[:, m * H:(m + 1) * H],
                                         pT_t[:, sl], start=True, stop=True)
                        nc.tensor.matmul(ps_c[:], wp2_t[:, m * H:(m + 1) * H],
                                         p2T_t[:, sl], start=True, stop=True)
                        dg = sview(dst, sl, "g")
                        dc = sview(dst, sl, "c")
                        nc.scalar.activation(dg, ps_g[:],
                                             mybir.ActivationFunctionType.Copy)
                        nc.vector.tensor_tensor(out=dc, in0=dg, in1=ps_c[:],
                                                op=mybir.AluOpType.add)

                for s in ("g", "c"):
                    resblock(s, 0, net_i, pooled_i)

                for i in range(1, NB):
                    # Launch all three strip gathers up front: ap_gather is
                    # async on the Q7s (~0.5us launch, ~2-30ns/token drain),
                    # so independent gathers overlap when queued together.
                    strips = []
                    for pl in range(NPLANES):
                        strip = stripp.tile([128, W[pl], 2], dt.bfloat16,
                                            tag="strip", name=f"strip{pl}")
                        nc.gpsimd.ap_gather(strip[:], net_i[:], smax_t[pl][:],
                                            128, TP, 2, W[pl])
                        strips.append(strip)
                    for pl in range(NPLANES):
                        strip = strips[pl]
                        for j, r in enumerate(range(2, RMAX[pl] + 1)):
                            wr = WR[pl][j + 1]
                            o = int(OFF[pl][j + 1])
                            nc.vector.tensor_tensor(
                                out=strip[:, :wr, :], in0=strip[:, :wr, :],
                                in1=strip[:, o:o + wr, :], op=mybir.AluOpType.max)
                        if pl == 0:
                            nc.gpsimd.ap_gather(pooled_i[:], strip[:, :N1[pl], :],
                                                pidx_t[pl][:], 128, N1[pl], 2, T)
                        else:
                            for q in range(4):
                                qsl = slice(q * 2048, (q + 1) * 2048)
                                exp = expp.tile([128, 2048, 2], dt.bfloat16,
                                                tag="exp", name=f"exp{pl}_{q}")
                                nc.gpsimd.ap_gather(
                                    exp[:], strip[:, :N1[pl], :],
                                    pidx_t[pl][:, q * 128:(q + 1) * 128],
                                    128, N1[pl], 2, 2048)
                                nc.vector.tensor_tensor(
                                    out=pooled_i[:, qsl, :],
                                    in0=pooled_i[:, qsl, :],
                                    in1=exp[:], op=mybir.AluOpType.add)
                    for s in ("g", "c"):
                        resblock(s, i, net_i, pooled_i)

                # ---- mean stage ----
                for pl in range(NPLANES):
                    strip = stripp.tile([128, W[pl], 2], dt.bfloat16,
                                        tag="strip", name=f"mstrip{pl}")
                    nc.gpsimd.ap_gather(strip[:], net_i[:], ssum_t[pl][:],
                                        128, TP, 2, W[pl])
                    acc = poolp.tile([128, N1[pl], 2], dt.float32, tag="pool",
                                     name=f"acc{pl}")
                    nc.vector.tensor_copy(acc[:], strip[:, :N1[pl], :])
                    for j, r in enumerate(range(2, RMAX[pl] + 1)):
                        wr = WR[pl][j + 1]
                        o = int(OFF[pl][j + 1])
                        nc.vector.tensor_tensor(
                            out=acc[:, :wr, :], in0=acc[:, :wr, :],
                            in1=strip[:, o:o + wr, :], op=mybir.AluOpType.add)
                    accb = stripp.tile([128, 2, N1[pl]], dt.bfloat16, tag="strip",
                                       name=f"accb{pl}")
                    for s in ("g", "c"):
                        si = SI[s]
                        nc.vector.tensor_copy(
                            accb[:, si, :],
                            acc[:, :, si:si + 1].rearrange("p t s -> p (t s)"))
                    nch = N1[pl] // 128
                    sums = stripp.tile([128, nch, 2 * C], dt.float32, tag="strip",
                                       name=f"sums{pl}")
                    for k in range(nch):
                        pb = psump.tile([128, 2 * C], dt.float32, tag="ph", name="pb")
                        ksl = slice(k * 128, (k + 1) * 128)
                        nc.tensor.matmul(pb[:, :C], accb[:, 0, ksl], Wt["g"]["fcw"][:],
                                         start=True, stop=True)
                        nc.tensor.matmul(pb[:, C:], accb[:, 1, ksl], Wt["c"]["fcw"][:],
                                         start=True, stop=True)
                        nc.vector.tensor_copy(sums[:, k, :], pb[:])
                    nc.gpsimd.dma_scatter_add(
                        out_d[pl][:], sums[:], sbin_t[pl][:],
                        N1[pl], N1[pl], 2 * C, single_packet=False)

            if timing:
                chk_t = constp.tile([128, 128], dt.bfloat16)
                nc.vector.tensor_copy(chk_t[:], net_i[:, :64, :].rearrange(
                    "p t s -> p (t s)"))
                nc.sync.dma_start(chk_d[:], chk_t[:])

    nc.compile()

    # ---- per-core input maps ----
    in_maps = []
    for b in range(B):
        im = {
            "pT": np.ascontiguousarray(p[b].T).astype(BF),
            "p2T": np.ascontiguousarray(p2[b].T).astype(BF),
            "wp": wp.astype(BF), "wp2": wp2.astype(BF),
        }
        for s in ("g", "c"):
            sh = sh_host[s]
            w0pk = np.concatenate([sh["w0"][:, :H].transpose(1, 0, 2),
                                   sh["w0"][:, H:].transpose(1, 0, 2)], axis=2)
            wspk = np.concatenate([sh["ws"][:, :H].transpose(1, 0, 2),
                                   sh["ws"][:, H:].transpose(1, 0, 2)], axis=2)
            w1pk = sh["w1"].transpose(1, 0, 2)
            rb = np.zeros((H, NB, 2), F32)
            for i, (ba, bb) in enumerate(sh["relu_bias"]):
                rb[:, i, 0] = ba
                rb[:, i, 1] = bb
            im[f"{s}_w0"] = np.ascontiguousarray(w0pk).astype(BF)
            im[f"{s}_w1"] = np.ascontiguousarray(w1pk).astype(BF)
            im[f"{s}_ws"] = np.ascontiguousarray(wspk).astype(BF)
            im[f"{s}_rb"] = rb
            im[f"{s}_b0"] = np.ascontiguousarray(sh["b0"].T).astype(F32)
            im[f"{s}_fcw"] = fc_w[s].astype(BF)
        for pl in range(NPLANES):
            pr = preps[b][pl]
            smax = np.concatenate(
                [pr.round_ids(r, WR[pl][r - 1], zero_pad=False)
                 for r in range(1, RMAX[pl] + 1)])
            ssum = np.concatenate(
                [pr.round_ids(r, WR[pl][r - 1], zero_pad=True)
                 for r in range(1, RMAX[pl] + 1)])
            im[f"smax_{pl}"] = wrap_idxs(smax)
            im[f"ssum_{pl}"] = wrap_idxs(ssum)
            im[f"pidx_{pl}"] = wrap_idxs(pr.pidx)
            empty = np.where(pr.cnt == 0)[0]
            sb = np.full(N1[pl], int(empty[0]) if len(empty) else 0, np.int64)
            sb[:pr.n_occ] = pr.bins_sorted
            im[f"sbin_{pl}"] = wrap_idxs(sb)
        in_maps.append(im)

    return nc, in_maps, cvec


def kernel(**inputs):
    from concourse.bass_utils import run_bass_kernel_spmd

    preps = _prep(inputs)
    nc, in_maps, cvec = _build(inputs, preps, REPS=1, timing=False)
    res = run_bass_kernel_spmd(nc, in_maps, core_ids=list(range(B)))

    out = np.zeros((2 * NPLANES, B, C, R, R), F32)
    for b in range(B):
        for pl in range(NPLANES):
            grid = np.asarray(res.results[b][f"out_{pl}"], F32)
            pr = preps[b][pl]
            cnt = pr.cnt.astype(F32)
            for si, s in enumerate(("g", "c")):
                part = grid[:, si * C:(si + 1) * C]
                true_sums = part + cnt[:, None] * cvec[s][None, :]
                mean = true_sums / np.clip(cnt, 1.0, None)[:, None]
                mean[cnt == 0] = 0.0
                out[si * NPLANES + pl, b] = mean.T.reshape(C, R, R)
    return out


if __name__ == "__main__":
    import reference
    inputs = {k: np.asarray(v) for k, v in reference.setup_inputs().items()}
    result = kernel(**inputs)
    print("kernel output shape:", result.shape)


# revision 6
# speedup vs baseline: 77.4831x; 1.2966x over previous
"""TRN2 Bass kernel for nn_LocalPoolPointnetPPFusion (batch-parallel, 8 cores).

Per-core pipeline (feature-major activations [128, 8192] bf16, biases deferred):
  net0' = p @ wp (+ p2 @ wp2 for corr stream)         (biases deferred to host)
  5 resblocks per stream (in-place, bf16 matmuls, fp32 PSUM); between blocks:
    net_fm --xbar--> net_pm [128, 65, 128] (chunk 64 = zeros, stays in SBUF)
    per plane: SBUF-source transpose-gathers build occupancy-sorted FM strips,
    prefix TT-max -> per-bin max (FM) --xbar--> table_pm -> SBUF-source
    transpose-gather expands to pooled' FM; 3 planes summed.
  final stage: same strips with fp32 prefix TT-add (per-bin sums of net'),
    cast bf16, @ fc_w on PE -> PM fp32 -> collision-free dma_scatter_add into
    zero-donated output grids [R*R, C] (HBM only here).
  host folds all deferred biases + fc bias + 1/cnt + transposes to [C, R, R].
"""
import sys
sys.path.insert(0, "/opt/trn_rl_repo")

import numpy as np
import ml_dtypes

BF = ml_dtypes.bfloat16
F32 = np.float32

B, T, H, C, R = 8, 8192, 128, 128, 128
NB = 5
NPLANES = 3
PLANE_COLS = ((0, 2), (0, 1), (1, 2))
ZROW = T          # zero-token index (chunk 64 of net_pm)
CHUNK = 2048      # mean-stage slot chunk
SCHUNK = 1024     # scatter chunk (PM sums tile)


def compute_idx_lists(p_np):
    import jax
    import jax.numpy as jnp
    cpu = jax.devices("cpu")[0]
    out = []
    with jax.default_device(cpu):
        pj = jnp.asarray(p_np)
        for cols in PLANE_COLS:
            xy = pj[..., jnp.array(cols)] / (1.0 + 0.0 + 1e-3) + 0.5
            xy = jnp.clip(xy, 0.0, 1.0 - 1e-3)
            g = jnp.floor(xy * R).astype(jnp.int32)
            out.append(np.asarray(g[..., 0] + R * g[..., 1]))
    return out


def wrap_idxs(flat):
    """token i -> idxs[i%16, i//16]; replicated to 128 partitions."""
    flat = np.asarray(flat, np.int64)
    n = len(flat)
    assert n % 16 == 0
    a = np.zeros((16, n // 16), np.int16)
    for i in range(n):
        a[i % 16, i // 16] = flat[i]
    return np.tile(a, (8, 1))


def ceil128(x):
    return max((int(x) + 127) // 128 * 128, 128)


class PlanePrep:
    def __init__(self, idx):
        self.idx = idx
        cnt = np.bincount(idx, minlength=R * R)
        self.cnt = cnt
        occ = np.where(cnt > 0)[0]
        order = np.argsort(-cnt[occ], kind="stable")
        self.bins_sorted = occ[order]
        self.n_occ = len(occ)
        self.occ_sorted = cnt[self.bins_sorted]
        sort_by_bin = np.argsort(idx, kind="stable")
        starts = np.searchsorted(idx[sort_by_bin], self.bins_sorted)
        self.members = [sort_by_bin[s:s + k] for s, k in zip(starts, self.occ_sorted)]
        slot_of_bin = np.full(R * R, -1, np.int64)
        slot_of_bin[self.bins_sorted] = np.arange(self.n_occ)
        self.pidx = slot_of_bin[idx]
        self.R_max = int(self.occ_sorted[0])
        self.n_r = [int((self.occ_sorted >= r).sum()) for r in range(1, self.R_max + 1)]

    def nr(self, r):
        return self.n_r[r - 1] if r <= self.R_max else 0

    def round_ids(self, r, width, sum_pad):
        ids = np.full(width, ZROW, np.int64)
        nr = self.nr(r)
        for s in range(min(nr, width)):
            ids[s] = self.members[s][r - 1]
        if not sum_pad:
            for s in range(nr, width):
                ids[s] = self.members[s][0] if s < self.n_occ else ZROW
        return ids


def _build(inputs, preps, REPS=1, timing=False):
    """Build program + per-core in_maps. timing=True uses internal grids."""
    import concourse.bacc as bacc
    import concourse.tile as tile
    from concourse import mybir

    p = np.asarray(inputs["p"], F32)
    p2 = np.asarray(inputs["p2"], F32)

    N1P = [max(ceil128(preps[b][pl].n_occ) for b in range(B)) for pl in range(NPLANES)]
    RMAX = [max(preps[b][pl].R_max for b in range(B)) for pl in range(NPLANES)]
    CR = []
    for pl in range(NPLANES):
        CR.append([ceil128(max(preps[b][pl].nr(r) for b in range(B)))
                   for r in range(2, RMAX[pl] + 1)])
    MAXCR = max(max(c) if c else 128 for c in CR)
    MAXN1P = max(N1P)

    def stream_host(pref, base_bias):
        w0 = np.asarray(inputs[f"{pref}_w0"], F32)
        b0 = np.asarray(inputs[f"{pref}_b0"], F32)
        w1 = np.asarray(inputs[f"{pref}_w1"], F32)
        b1 = np.asarray(inputs[f"{pref}_b1"], F32)
        ws = np.asarray(inputs[f"{pref}_ws"], F32)
        relu_bias = []
        Bp = base_bias
        for i in range(NB):
            if i == 0:
                bias_in = Bp
                relu_bias.append((bias_in[:H].copy(), bias_in[H:].copy()))
            else:
                bias_in = np.concatenate([Bp, 3.0 * Bp])
                relu_bias.append((Bp.copy(), 3.0 * Bp))
            Bp = b1[i] + bias_in @ ws[i]
        return dict(w0=w0, b0=b0, w1=w1, ws=ws, relu_bias=relu_bias, B_final=Bp)

    wp = np.asarray(inputs["wp"], F32)
    bp = np.asarray(inputs["bp"], F32)
    wp2 = np.asarray(inputs["wp2"], F32)
    bp2 = np.asarray(inputs["bp2"], F32)
    sh_host = {"g": stream_host("blk", bp.copy()), "c": stream_host("blkc", bp + bp2)}
    fc_w = {"g": np.asarray(inputs["fc_c_w"], F32),
            "c": np.asarray(inputs["fc_cc_w"], F32)}
    fc_b = {"g": np.asarray(inputs["fc_c_b"], F32),
            "c": np.asarray(inputs["fc_cc_b"], F32)}
    cvec = {s: sh_host[s]["B_final"] @ fc_w[s] + fc_b[s] for s in ("g", "c")}

    nc = bacc.Bacc("TRN2", target_bir_lowering=False, debug=False, num_devices=B)
    dt = mybir.dt

    def din(name, shape, dtype):
        return nc.dram_tensor(name, shape, dtype, kind="ExternalInput")

    pT_d = din("pT", [3, T], dt.bfloat16)
    p2T_d = din("p2T", [3, T], dt.bfloat16)
    wp_d = din("wp", [3, 2 * H], dt.bfloat16)
    wp2_d = din("wp2", [3, 2 * H], dt.bfloat16)
    wpk_d = {}
    for s in ("g", "c"):
        wpk_d[s] = dict(
            w0=din(f"{s}_w0", [H, NB, 2 * H], dt.bfloat16),
            w1=din(f"{s}_w1", [H, NB, H], dt.bfloat16),
            ws=din(f"{s}_ws", [H, NB, 2 * H], dt.bfloat16),
            rb=din(f"{s}_rb", [H, NB, 2], dt.float32),
            b0=din(f"{s}_b0", [H, NB], dt.float32),
            fcw=din(f"{s}_fcw", [H, C], dt.bfloat16),
        )
    g1_d = [din(f"g1_{pl}", [128, N1P[pl] // 16], dt.int16) for pl in range(NPLANES)]
    gmax_d = [[din(f"gmax_{pl}_{r}", [128, CR[pl][r - 2] // 16], dt.int16)
               for r in range(2, RMAX[pl] + 1)] for pl in range(NPLANES)]
    gsum_d = [[din(f"gsum_{pl}_{r}", [128, CR[pl][r - 2] // 16], dt.int16)
               for r in range(2, RMAX[pl] + 1)] for pl in range(NPLANES)]
    pidx_d = [din(f"pidx_{pl}", [128, T // 16], dt.int16) for pl in range(NPLANES)]
    sbin_d = [din(f"sbin_{pl}", [128, N1P[pl] // 16], dt.int16) for pl in range(NPLANES)]

    out_kind = "Internal" if timing else "ExternalOutput"
    out_d = {(s, pl): nc.dram_tensor(f"out_{s}{pl}", [R * R, C], dt.float32,
                                     kind=out_kind)
             for s in ("g", "c") for pl in range(NPLANES)}
    chk_d = nc.dram_tensor("chk", [128, 128], dt.bfloat16, kind="ExternalOutput") \
        if timing else None

    with tile.TileContext(nc) as tc:
        with tc.tile_pool(name="const", bufs=1) as constp, \
             tc.tile_pool(name="act", bufs=1) as actp, \
             tc.tile_pool(name="pooledp", bufs=2) as pooledp, \
             tc.tile_pool(name="small", bufs=3) as smallp, \
             tc.tile_pool(name="sr", bufs=3) as srp, \
             tc.tile_pool(name="gp", bufs=1) as gp, \
             tc.tile_pool(name="npm", bufs=2) as npmp, \
             tc.tile_pool(name="pm", bufs=2) as pmp, \
             tc.tile_pool(name="psum", bufs=2, space="PSUM") as psump:

            wp_t = constp.tile([3, 2 * H], dt.bfloat16)
            wp2_t = constp.tile([3, 2 * H], dt.bfloat16)
            nc.sync.dma_start(wp_t[:], wp_d[:])
            nc.sync.dma_start(wp2_t[:], wp2_d[:])
            W = {}
            for s in ("g", "c"):
                W[s] = dict(
                    w0=constp.tile([H, NB, 2 * H], dt.bfloat16, tag=f"{s}w0", name=f"{s}w0"),
                    w1=constp.tile([H, NB, H], dt.bfloat16, tag=f"{s}w1", name=f"{s}w1"),
                    ws=constp.tile([H, NB, 2 * H], dt.bfloat16, tag=f"{s}ws", name=f"{s}ws"),
                    rb=constp.tile([H, NB, 2], dt.float32, tag=f"{s}rb", name=f"{s}rb"),
                    b0=constp.tile([H, NB], dt.float32, tag=f"{s}b0", name=f"{s}b0"),
                    fcw=constp.tile([H, C], dt.bfloat16, tag=f"{s}fcw", name=f"{s}fcw"),
                )
                for k, t in W[s].items():
                    nc.sync.dma_start(t[:], wpk_d[s][k][:])
            g1_t, gmax_t, gsum_t, pidx_t, sbin_t = [], [], [], [], []
            for pl in range(NPLANES):
                g1_t.append(constp.tile([128, N1P[pl] // 16], dt.int16,
                                        tag=f"g1{pl}", name=f"g1t{pl}"))
                pidx_t.append(constp.tile([128, T // 16], dt.int16,
                                          tag=f"pi{pl}", name=f"pit{pl}"))
                sbin_t.append(constp.tile([128, N1P[pl] // 16], dt.int16,
                                          tag=f"sb{pl}", name=f"sbt{pl}"))
                nc.sync.dma_start(g1_t[pl][:], g1_d[pl][:])
                nc.sync.dma_start(pidx_t[pl][:], pidx_d[pl][:])
                nc.sync.dma_start(sbin_t[pl][:], sbin_d[pl][:])
                gm, gs = [], []
                for j in range(RMAX[pl] - 1):
                    tm = constp.tile([128, CR[pl][j] // 16], dt.int16,
                                     tag=f"gm{pl}_{j}", name=f"gmt{pl}_{j}")
                    ts_ = constp.tile([128, CR[pl][j] // 16], dt.int16,
                                      tag=f"gs{pl}_{j}", name=f"gst{pl}_{j}")
                    nc.sync.dma_start(tm[:], gmax_d[pl][j][:])
                    nc.sync.dma_start(ts_[:], gsum_d[pl][j][:])
                    gm.append(tm)
                    gs.append(ts_)
                gmax_t.append(gm)
                gsum_t.append(gs)

            def sbuf_gather(dst_ap, src_pm, idxs_ap, n):
                """SBUF-source transpose gather: token i at [i%128, i//128, :]."""
                nc.gpsimd.dma_gather(
                    dst_ap, src_pm, idxs_ap, n, n, H,
                    transpose=True, single_packet=False,
                    sbuf_tokens_per_rank=128,
                    sbuf_free_dim_per_rank=H * 2,
                )

            def make_net_pm(s, net_fm):
                """Transpose net' into PM [128, 65, 128]; chunk 64 = zeros."""
                npm = npmp.tile([128, 65, H], dt.bfloat16, tag="npm", name="npm")
                nc.vector.memset(npm[:, 64, :], 0.0)
                nc.sync.dma_start_transpose(npm[:, :64, :], net_fm[:])
                return npm

            def pool_local(s, npm):
                pooled = pooledp.tile([H, T], dt.bfloat16, tag="pooled", name="pooled")
                for pl in range(NPLANES):
                    n1 = N1P[pl]
                    s1 = pmp.tile([128, 1, MAXN1P], dt.bfloat16, tag="pm", name="s1")
                    sbuf_gather(s1[:, :, :n1], npm[:], g1_t[pl][:], n1)
                    for j in range(RMAX[pl] - 1):
                        w = CR[pl][j]
                        sr = srp.tile([128, 1, MAXCR], dt.bfloat16, tag="sr", name="sr")
                        sbuf_gather(sr[:, :, :w], npm[:], gmax_t[pl][j][:], w)
                        nc.vector.tensor_tensor(
                            out=s1[:, 0, :w], in0=s1[:, 0, :w],
                            in1=sr[:, 0, :w], op=mybir.AluOpType.max)
                    tbl = pmp.tile([128, MAXN1P // 128, H], dt.bfloat16,
                                   tag="pm", name="tbl")
                    nc.sync.dma_start_transpose(tbl[:, :n1 // 128, :], s1[:, 0, :n1])
                    if pl == 0:
                        sbuf_gather(pooled[:].rearrange("h (a t) -> h a t", a=1),
                                    tbl[:], pidx_t[pl][:], T)
                    else:
                        g = gp.tile([128, 1, T], dt.bfloat16, tag="g", name="g")
                        sbuf_gather(g[:], tbl[:], pidx_t[pl][:], T)
                        nc.vector.tensor_tensor(out=pooled[:], in0=pooled[:],
                                                in1=g[:, 0, :], op=mybir.AluOpType.add)
                return pooled

            def resblock(s, i, xa, xb):
                """In-place: writes output into xa. Returns xa."""
                w = W[s]
                ba_ap = w["rb"][:, i, 0:1]
                bb_ap = w["rb"][:, i, 1:2]
                for nt in range(T // 512):
                    sl = slice(nt * 512, (nt + 1) * 512)
                    ra = smallp.tile([H, 512], dt.bfloat16, tag="ra", name="ra")
                    rb_ = smallp.tile([H, 512], dt.bfloat16, tag="rb", name="rb")
                    nc.vector.tensor_scalar(out=ra[:], in0=xa[:, sl], scalar1=ba_ap,
                                            scalar2=0.0, op0=mybir.AluOpType.add,
                                            op1=mybir.AluOpType.max)
                    nc.vector.tensor_scalar(out=rb_[:], in0=xb[:, sl], scalar1=bb_ap,
                                            scalar2=0.0, op0=mybir.AluOpType.add,
                                            op1=mybir.AluOpType.max)
                    ph = psump.tile([H, 512], dt.float32, tag="ph", name="ph")
                    nc.tensor.matmul(ph[:], w["w0"][:, i, :H], ra[:],
                                     start=True, stop=False)
                    nc.tensor.matmul(ph[:], w["w0"][:, i, H:], rb_[:],
                                     start=False, stop=True)
                    h = smallp.tile([H, 512], dt.bfloat16, tag="h", name="h")
                    nc.scalar.activation(h[:], ph[:], mybir.ActivationFunctionType.Relu,
                                         bias=w["b0"][:, i:i + 1], scale=1.0)
                    po = psump.tile([H, 512], dt.float32, tag="po", name="po")
                    nc.tensor.matmul(po[:], w["w1"][:, i, :], h[:],
                                     start=True, stop=False)
                    nc.tensor.matmul(po[:], w["ws"][:, i, :H], xa[:, sl],
                                     start=False, stop=False)
                    nc.tensor.matmul(po[:], w["ws"][:, i, H:], xb[:, sl],
                                     start=False, stop=True)
                    nc.scalar.activation(xa[:, sl], po[:],
                                         mybir.ActivationFunctionType.Copy)
                return xa

            def mean_stage(s, npm):
                for pl in range(NPLANES):
                    n1 = N1P[pl]
                    for c0 in range(0, n1, CHUNK):
                        wch = min(CHUNK, n1 - c0)
                        s1f = srp.tile([128, 1, CHUNK], dt.bfloat16, tag="sr", name="s1f")
                        sbuf_gather(s1f[:, :, :wch], npm[:],
                                    g1_t[pl][:, c0 // 16:(c0 + wch) // 16], wch)
                        acc = pmp.tile([H, CHUNK], dt.float32, tag="pm", name="acc")
                        nc.vector.tensor_copy(acc[:, :wch], s1f[:, 0, :wch])
                        for j in range(RMAX[pl] - 1):
                            w = min(CR[pl][j], c0 + wch) - c0
                            if w <= 0:
                                continue
                            srf = srp.tile([128, 1, CHUNK], dt.bfloat16,
                                           tag="sr", name="srf")
                            sbuf_gather(srf[:, :, :w], npm[:],
                                        gsum_t[pl][j][:, c0 // 16:(c0 + w) // 16], w)
                            srf32 = gp.tile([H, CHUNK], dt.float32, tag="g", name="srf32")
                            nc.vector.tensor_copy(srf32[:, :w], srf[:, 0, :w])
                            nc.vector.tensor_tensor(out=acc[:, :w], in0=acc[:, :w],
                                                    in1=srf32[:, :w],
                                                    op=mybir.AluOpType.add)
                        accb = srp.tile([128, 1, CHUNK], dt.bfloat16,
                                        tag="sr", name="accb")
                        nc.vector.tensor_copy(accb[:, 0, :wch], acc[:, :wch])
                        for sc0 in range(0, wch, SCHUNK):
                            wsc = min(SCHUNK, wch - sc0)
                            sums = srp.tile([128, SCHUNK // 128, C], dt.float32,
                                            tag="sums", name="sums", bufs=1)
                            for ch4 in range((wsc // 128 + 3) // 4):
                                pb = psump.tile([128, 512], dt.float32,
                                                tag="ph", name="pb")
                                nch = min(4, wsc // 128 - ch4 * 4)
                                for k in range(nch):
                                    chunk = ch4 * 4 + k
                                    nc.tensor.matmul(
                                        pb[:, k * C:(k + 1) * C],
                                        accb[:, 0, sc0 + chunk * 128:
                                             sc0 + (chunk + 1) * 128],
                                        W[s]["fcw"][:], start=True, stop=True)
                                nc.vector.tensor_copy(
                                    sums[:, ch4 * 4:ch4 * 4 + nch, :].rearrange(
                                        "p a f -> p (a f)"),
                                    pb[:, :nch * C])
                            nc.gpsimd.dma_scatter_add(
                                out_d[(s, pl)][:], sums[:, :wsc // 128, :],
                                sbin_t[pl][:, (c0 + sc0) // 16:(c0 + sc0 + wsc) // 16],
                                wsc, wsc, C, single_packet=False)

            # ---------------- schedule ----------------
            net = {}
            for rep in range(REPS):
                pT_t = npmp.tile([3, T], dt.bfloat16, tag="npm", name="pT_t")
                p2T_t = npmp.tile([3, T], dt.bfloat16, tag="npm", name="p2T_t")
                nc.sync.dma_start(pT_t[:], pT_d[:])
                nc.sync.dma_start(p2T_t[:], p2T_d[:])
                x0 = {"g": [actp.tile([H, T], dt.bfloat16, tag="netg", name="x0g0"),
                            pooledp.tile([H, T], dt.bfloat16, tag="pooled", name="x0g1")],
                      "c": [actp.tile([H, T], dt.bfloat16, tag="netc", name="x0c0"),
                            pooledp.tile([H, T], dt.bfloat16, tag="pooled", name="x0c1")]}
                for m in range(2):
                    for nt in range(T // 512):
                        sl = slice(nt * 512, (nt + 1) * 512)
                        ps_g = psump.tile([H, 512], dt.float32, tag="ph", name="ps_g")
                        ps_c = psump.tile([H, 512], dt.float32, tag="po", name="ps_c")
                        nc.tensor.matmul(ps_g[:], wp_t[:, m * H:(m + 1) * H],
                                         pT_t[:, sl], start=True, stop=True)
                        nc.tensor.matmul(ps_c[:], wp2_t[:, m * H:(m + 1) * H],
                                         p2T_t[:, sl], start=True, stop=True)
                        nc.scalar.activation(x0["g"][m][:, sl], ps_g[:],
                                             mybir.ActivationFunctionType.Copy)
                        nc.vector.tensor_tensor(out=x0["c"][m][:, sl],
                                                in0=x0["g"][m][:, sl],
                                                in1=ps_c[:], op=mybir.AluOpType.add)

                for s in ("g", "c"):
                    net[s] = resblock(s, 0, x0[s][0], x0[s][1])
                for i in range(1, NB):
                    npm = {}
                    for s in ("g", "c"):
                        npm[s] = make_net_pm(s, net[s])
                    pooled = {}
                    for s in ("g", "c"):
                        pooled[s] = pool_local(s, npm[s])
                    for s in ("g", "c"):
                        net[s] = resblock(s, i, net[s], pooled[s])
                for s in ("g", "c"):
                    npm_f = make_net_pm(s, net[s])
                    mean_stage(s, npm_f)

            if timing:
                chk_t = constp.tile([128, 128], dt.bfloat16)
                nc.vector.tensor_copy(chk_t[:], net["g"][:, :128])
                nc.sync.dma_start(chk_d[:], chk_t[:])

    nc.compile()

    in_maps = []
    for b in range(B):
        im = {
            "pT": np.ascontiguousarray(p[b].T).astype(BF),
            "p2T": np.ascontiguousarray(p2[b].T).astype(BF),
            "wp": wp.astype(BF), "wp2": wp2.astype(BF),
        }
        for s in ("g", "c"):
            sh = sh_host[s]
            w0pk = np.concatenate([sh["w0"][:, :H].transpose(1, 0, 2),
                                   sh["w0"][:, H:].transpose(1, 0, 2)], axis=2)
            wspk = np.concatenate([sh["ws"][:, :H].transpose(1, 0, 2),
                                   sh["ws"][:, H:].transpose(1, 0, 2)], axis=2)
            w1pk = sh["w1"].transpose(1, 0, 2)
            rb = np.zeros((H, NB, 2), F32)
            for i, (ba, bb) in enumerate(sh["relu_bias"]):
                rb[:, i, 0] = ba
                rb[:, i, 1] = bb
            im[f"{s}_w0"] = np.ascontiguousarray(w0pk).astype(BF)
            im[f"{s}_w1"] = np.ascontiguousarray(w1pk).astype(BF)
            im[f"{s}_ws"] = np.ascontiguousarray(wspk).astype(BF)
            im[f"{s}_rb"] = rb
            im[f"{s}_b0"] = np.ascontiguousarray(sh["b0"].T).astype(F32)
            im[f"{s}_fcw"] = fc_w[s].astype(BF)
        for pl in range(NPLANES):
            pr = preps[b][pl]
            im[f"g1_{pl}"] = wrap_idxs(pr.round_ids(1, N1P[pl], sum_pad=True))
            for j, r in enumerate(range(2, RMAX[pl] + 1)):
                im[f"gmax_{pl}_{r}"] = wrap_idxs(pr.round_ids(r, CR[pl][j], sum_pad=False))
                im[f"gsum_{pl}_{r}"] = wrap_idxs(pr.round_ids(r, CR[pl][j], sum_pad=True))
            im[f"pidx_{pl}"] = wrap_idxs(pr.pidx)
            empty = np.where(pr.cnt == 0)[0]
            sb = np.full(N1P[pl], int(empty[0]) if len(empty) else 0, np.int64)
            sb[:pr.n_occ] = pr.bins_sorted
            im[f"sbin_{pl}"] = wrap_idxs(sb)
        in_maps.append(im)

    return nc, in_maps, cvec


def _prep(inputs):
    p = np.asarray(inputs["p"], F32)
    idx_lists = compute_idx_lists(p)
    return [[PlanePrep(idx_lists[pl][b]) for pl in range(NPLANES)] for b in range(B)]


def kernel(**inputs):
    from concourse.bass_utils import run_bass_kernel_spmd

    preps = _prep(inputs)
    nc, in_maps, cvec = _build(inputs, preps, REPS=1, timing=False)
    res = run_bass_kernel_spmd(nc, in_maps, core_ids=list(range(B)))

    out = np.zeros((2 * NPLANES, B, C, R, R), F32)
    for b in range(B):
        for si, s in enumerate(("g", "c")):
            for pl in range(NPLANES):
                grid = np.asarray(res.results[b][f"out_{s}{pl}"], F32)
                pr = preps[b][pl]
                cnt = pr.cnt.astype(F32)
                true_sums = grid + cnt[:, None] * cvec[s][None, :]
                mean = true_sums / np.clip(cnt, 1.0, None)[:, None]
                mean[cnt == 0] = 0.0
                out[si * NPLANES + pl, b] = mean.T.reshape(C, R, R)
    return out


def measure_hw_time(inputs, reps=8, n_timing_runs=6):
    """Estimate per-iteration device time via in-kernel repetition differencing."""
    import time
    from concourse.bass_utils import run_bass_kernel_spmd

    preps = _prep(inputs)

    def runner(R_):
        nc, in_maps, _ = _build(inputs, preps, REPS=R_, timing=True)

        def once():
            t0 = time.perf_counter()
            run_bass_kernel_spmd(nc, in_maps, core_ids=list(range(B)))
            return time.perf_counter() - t0
        once()  # warm
        return min(once() for _ in range(n_timing_runs))

    t1 = runner(1)
    tR = runner(reps)
    per_iter = (tR - t1) / (reps - 1)
    return int(per_iter * 1e9), t1, tR


if __name__ == "__main__":
    import reference
    inputs = {k: np.asarray(v) for k, v in reference.setup_inputs().items()}
    result = kernel(**inputs)
    print("kernel output shape:", result.shape)



# revision 7
# speedup vs baseline: 122.2529x; 1.5778x over previous
"""TRN2 Bass kernel for nn_LocalPoolPointnetPPFusion (batch-parallel, 8 cores).

v3 = v1's dma_gather mechanism (7.6ns/idx Q7 desc-gen, engine-blocking) with
v2's op structure: both streams packed per token (512B payloads, one gather
serves g+c), all scatter-max rounds merged into ONE strip gather per plane
(DVE tensor_tensor maxes on strip segments), raised SWDGE ring (4096 descs)
so full-width gathers fit. ~62 dyn-DMA ops and ~232k descriptors per rep vs
832 ops / 518k descriptors in v1.
"""
import sys
sys.path.insert(0, "/opt/trn_rl_repo")

import numpy as np
import ml_dtypes

BF = ml_dtypes.bfloat16
F32 = np.float32

B, T, H, C, R = 8, 8192, 128, 128, 128
NB = 5
NPLANES = 3
PLANE_COLS = ((0, 2), (0, 1), (1, 2))
TZ = T            # zero token: npm rank 64 is memset to 0


def compute_idx_lists(p_np):
    import jax
    import jax.numpy as jnp
    cpu = jax.devices("cpu")[0]
    out = []
    with jax.default_device(cpu):
        pj = jnp.asarray(p_np)
        for cols in PLANE_COLS:
            xy = pj[..., jnp.array(cols)] / (1.0 + 0.0 + 1e-3) + 0.5
            xy = jnp.clip(xy, 0.0, 1.0 - 1e-3)
            g = jnp.floor(xy * R).astype(jnp.int32)
            out.append(np.asarray(g[..., 0] + R * g[..., 1]))
    return out


def wrap_idxs(flat):
    flat = np.asarray(flat, np.int64)
    n = len(flat)
    assert n % 16 == 0
    a = flat.reshape(n // 16, 16).T.astype(np.int16)
    return np.tile(a, (8, 1))


def align(x, a):
    return (int(x) + a - 1) // a * a


class PlanePrep:
    def __init__(self, idx):
        self.idx = idx
        cnt = np.bincount(idx, minlength=R * R)
        self.cnt = cnt
        occ = np.where(cnt > 0)[0]
        order = np.argsort(-cnt[occ], kind="stable")
        self.bins_sorted = occ[order]
        self.n_occ = len(occ)
        self.occ_sorted = cnt[self.bins_sorted]
        sort_by_bin = np.argsort(idx, kind="stable")
        starts = np.searchsorted(idx[sort_by_bin], self.bins_sorted)
        self.members = [sort_by_bin[s:s + k] for s, k in zip(starts, self.occ_sorted)]
        slot_of_bin = np.full(R * R, -1, np.int64)
        slot_of_bin[self.bins_sorted] = np.arange(self.n_occ)
        self.pidx = slot_of_bin[idx]
        self.R_max = int(self.occ_sorted[0])
        self.n_r = [int((self.occ_sorted >= r).sum()) for r in range(1, self.R_max + 1)]

    def nr(self, r):
        return self.n_r[r - 1] if r <= self.R_max else 0

    def round_ids(self, r, width, zero_pad):
        ids = np.full(width, TZ if zero_pad else int(self.members[0][0]), np.int64)
        nr = self.nr(r)
        for s in range(min(nr, width)):
            ids[s] = self.members[s][r - 1]
        if not zero_pad:
            for s in range(nr, width):
                if s < self.n_occ:
                    ids[s] = self.members[s][0]
        return ids


def _prep(inputs):
    p = np.asarray(inputs["p"], F32)
    idx_lists = compute_idx_lists(p)
    return [[PlanePrep(idx_lists[pl][b]) for pl in range(NPLANES)] for b in range(B)]


def _build(inputs, preps, REPS=1, timing=False):
    import concourse.bacc as bacc
    import concourse.tile as tile
    from concourse import mybir

    p = np.asarray(inputs["p"], F32)
    p2 = np.asarray(inputs["p2"], F32)

    # ---- strip geometry (shared across batch) ----
    RMAX = [max(preps[b][pl].R_max for b in range(B)) for pl in range(NPLANES)]
    WR, OFF, N1, W, WG = [], [], [], [], []
    for pl in range(NPLANES):
        wr = [align(max(preps[b][pl].n_occ for b in range(B)), 128)]
        for r in range(2, RMAX[pl] + 1):
            wr.append(align(max(preps[b][pl].nr(r) for b in range(B)), 16))
        off = np.concatenate([[0], np.cumsum(wr)])
        WR.append(wr)
        OFF.append(off)
        N1.append(wr[0])
        W.append(int(off[-1]))
        WG.append(align(int(off[-1]), 128))
    # split point: first round boundary >= W/2 (part A holds rounds 1..KS-1)
    KS, WA, WB = [], [], []
    for pl in range(NPLANES):
        k = next(j for j in range(1, len(OFF[pl])) if OFF[pl][j] >= W[pl] / 2)
        KS.append(k)
        WA.append(align(int(OFF[pl][k]), 128))
        WB.append(align(W[pl] - int(OFF[pl][k]), 128))

    # ---- host-side weight/bias folding (identical to v1) ----
    def stream_host(pref, base_bias):
        w0 = np.asarray(inputs[f"{pref}_w0"], F32)
        b0 = np.asarray(inputs[f"{pref}_b0"], F32)
        w1 = np.asarray(inputs[f"{pref}_w1"], F32)
        b1 = np.asarray(inputs[f"{pref}_b1"], F32)
        ws = np.asarray(inputs[f"{pref}_ws"], F32)
        relu_bias = []
        Bp = base_bias
        for i in range(NB):
            if i == 0:
                bias_in = Bp
                relu_bias.append((bias_in[:H].copy(), bias_in[H:].copy()))
            else:
                bias_in = np.concatenate([Bp, 3.0 * Bp])
                relu_bias.append((Bp.copy(), 3.0 * Bp))
            Bp = b1[i] + bias_in @ ws[i]
        return dict(w0=w0, b0=b0, w1=w1, ws=ws, relu_bias=relu_bias, B_final=Bp)

    wp = np.asarray(inputs["wp"], F32)
    bp = np.asarray(inputs["bp"], F32)
    wp2 = np.asarray(inputs["wp2"], F32)
    bp2 = np.asarray(inputs["bp2"], F32)
    sh_host = {"g": stream_host("blk", bp.copy()), "c": stream_host("blkc", bp + bp2)}
    fc_w = {"g": np.asarray(inputs["fc_c_w"], F32),
            "c": np.asarray(inputs["fc_cc_w"], F32)}
    fc_b = {"g": np.asarray(inputs["fc_c_b"], F32),
            "c": np.asarray(inputs["fc_cc_b"], F32)}
    cvec = {s: sh_host[s]["B_final"] @ fc_w[s] + fc_b[s] for s in ("g", "c")}

    nc = bacc.Bacc("TRN2", target_bir_lowering=False, debug=False, num_devices=B)
    dt = mybir.dt

    def din(name, shape, dtype):
        return nc.dram_tensor(name, shape, dtype, kind="ExternalInput")

    pT_d = din("pT", [3, T], dt.bfloat16)
    p2T_d = din("p2T", [3, T], dt.bfloat16)
    wp_d = din("wp", [3, 2 * H], dt.bfloat16)
    wp2_d = din("wp2", [3, 2 * H], dt.bfloat16)
    wpk_d = {}
    for s in ("g", "c"):
        wpk_d[s] = dict(
            w0=din(f"{s}_w0", [H, NB, 2 * H], dt.bfloat16),
            w1=din(f"{s}_w1", [H, NB, H], dt.bfloat16),
            ws=din(f"{s}_ws", [H, NB, 2 * H], dt.bfloat16),
            rb=din(f"{s}_rb", [H, NB, 2], dt.float32),
            b0=din(f"{s}_b0", [H, NB], dt.float32),
            fcw=din(f"{s}_fcw", [H, C], dt.bfloat16),
        )
    smax_d = [din(f"smax_{pl}", [128, (WA[pl] + WB[pl]) // 16], dt.int16) for pl in range(NPLANES)]
    ssum_d = [din(f"ssum_{pl}", [128, (WA[pl] + WB[pl]) // 16], dt.int16) for pl in range(NPLANES)]
    pidx_d = [din(f"pidx_{pl}", [128, T // 16], dt.int16) for pl in range(NPLANES)]
    sbin_d = [din(f"sbin_{pl}", [128, N1[pl] // 16], dt.int16) for pl in range(NPLANES)]

    out_kind = "Internal" if timing else "ExternalOutput"
    out_d = {pl: nc.dram_tensor(f"out_{pl}", [R * R, 2 * C], dt.float32, kind=out_kind)
             for pl in range(NPLANES)}
    chk_d = nc.dram_tensor("chk", [128, 128], dt.bfloat16, kind="ExternalOutput") \
        if timing else None

    SI = {"g": 0, "c": 1}

    with tile.TileContext(nc) as tc:
        with tc.tile_pool(name="const", bufs=1) as constp, \
             tc.tile_pool(name="act", bufs=1) as actp, \
             tc.tile_pool(name="npm", bufs=1) as npmp, \
             tc.tile_pool(name="pooled", bufs=1) as pooledp, \
             tc.tile_pool(name="strip", bufs=1) as stripp, \
             tc.tile_pool(name="tbl", bufs=2) as tblp, \
             tc.tile_pool(name="small", bufs=2) as smallp, \
             tc.tile_pool(name="psum", bufs=2, space="PSUM") as psump:

            wp_t = constp.tile([3, 2 * H], dt.bfloat16)
            wp2_t = constp.tile([3, 2 * H], dt.bfloat16)
            nc.sync.dma_start(wp_t[:], wp_d[:])
            nc.sync.dma_start(wp2_t[:], wp2_d[:])
            Wt = {}
            for s in ("g", "c"):
                Wt[s] = dict(
                    w0=constp.tile([H, NB, 2 * H], dt.bfloat16, tag=f"{s}w0", name=f"{s}w0"),
                    w1=constp.tile([H, NB, H], dt.bfloat16, tag=f"{s}w1", name=f"{s}w1"),
                    ws=constp.tile([H, NB, 2 * H], dt.bfloat16, tag=f"{s}ws", name=f"{s}ws"),
                    rb=constp.tile([H, NB, 2], dt.float32, tag=f"{s}rb", name=f"{s}rb"),
                    b0=constp.tile([H, NB], dt.float32, tag=f"{s}b0", name=f"{s}b0"),
                    fcw=constp.tile([H, C], dt.bfloat16, tag=f"{s}fcw", name=f"{s}fcw"),
                )
                for k, t in Wt[s].items():
                    nc.sync.dma_start(t[:], wpk_d[s][k][:])
            smax_t, ssum_t, pidx_t, sbin_t = [], [], [], []
            for pl in range(NPLANES):
                smax_t.append(constp.tile([128, (WA[pl] + WB[pl]) // 16], dt.int16,
                                          tag=f"sm{pl}", name=f"smt{pl}"))
                ssum_t.append(constp.tile([128, (WA[pl] + WB[pl]) // 16], dt.int16,
                                          tag=f"ss{pl}", name=f"sst{pl}"))
                pidx_t.append(constp.tile([128, T // 16], dt.int16,
                                          tag=f"pi{pl}", name=f"pit{pl}"))
                sbin_t.append(constp.tile([128, N1[pl] // 16], dt.int16,
                                          tag=f"sb{pl}", name=f"sbt{pl}"))
                nc.sync.dma_start(smax_t[pl][:], smax_d[pl][:])
                nc.sync.dma_start(ssum_t[pl][:], ssum_d[pl][:])
                nc.sync.dma_start(pidx_t[pl][:], pidx_d[pl][:])
                nc.sync.dma_start(sbin_t[pl][:], sbin_d[pl][:])

            def sbuf_gather(dst_ap, src, idxs_ap, n):
                nc.gpsimd.dma_gather(
                    dst_ap, src, idxs_ap, n, n, 2 * H,
                    transpose=True, single_packet=False,
                    sbuf_tokens_per_rank=128,
                    sbuf_free_dim_per_rank=4 * H,
                )

            def make_net_pm(net):
                """Transpose both streams into npm [128, 65, 256]; rank 64 = 0."""
                npm = npmp.tile([128, 65, 2 * H], dt.bfloat16, tag="npm", name="npm")
                nc.vector.memset(npm[:, 64, :], 0.0)
                nc.sync.dma_start_transpose(npm[:, :64, 0:H], net["g"][:])
                nc.sync.dma_start_transpose(npm[:, :64, H:2 * H], net["c"][:])
                return npm

            def resblock(s, i, xa, pooled):
                """xa: [H, T] tile (in-place). pooled: [128, 2, T] tile."""
                w = Wt[s]
                si = SI[s]
                ba_ap = w["rb"][:, i, 0:1]
                bb_ap = w["rb"][:, i, 1:2]
                for nt in range(T // 512):
                    sl = slice(nt * 512, (nt + 1) * 512)
                    xb = pooled[:, si, sl]
                    ra = smallp.tile([H, 512], dt.bfloat16, tag="ra", name="ra")
                    rb_ = smallp.tile([H, 512], dt.bfloat16, tag="rb", name="rb")
                    nc.vector.tensor_scalar(out=ra[:], in0=xa[:, sl], scalar1=ba_ap,
                                            scalar2=0.0, op0=mybir.AluOpType.add,
                                            op1=mybir.AluOpType.max)
                    nc.vector.tensor_scalar(out=rb_[:], in0=xb, scalar1=bb_ap,
                                            scalar2=0.0, op0=mybir.AluOpType.add,
                                            op1=mybir.AluOpType.max)
                    ph = psump.tile([H, 512], dt.float32, tag="ph", name="ph")
                    nc.tensor.matmul(ph[:], w["w0"][:, i, :H], ra[:],
                                     start=True, stop=False)
                    nc.tensor.matmul(ph[:], w["w0"][:, i, H:], rb_[:],
                                     start=False, stop=True)
                    h = smallp.tile([H, 512], dt.bfloat16, tag="h", name="h")
                    nc.scalar.activation(h[:], ph[:], mybir.ActivationFunctionType.Relu,
                                         bias=w["b0"][:, i:i + 1], scale=1.0)
                    po = psump.tile([H, 512], dt.float32, tag="po", name="po")
                    nc.tensor.matmul(po[:], w["w1"][:, i, :], h[:],
                                     start=True, stop=False)
                    nc.tensor.matmul(po[:], w["ws"][:, i, :H], xa[:, sl],
                                     start=False, stop=False)
                    nc.tensor.matmul(po[:], w["ws"][:, i, H:], xb,
                                     start=False, stop=True)
                    nc.scalar.activation(xa[:, sl], po[:],
                                         mybir.ActivationFunctionType.Copy)

            # ---------------- schedule ----------------
            for rep in range(REPS):
                net = {"g": actp.tile([H, T], dt.bfloat16, tag="netg", name="netg"),
                       "c": actp.tile([H, T], dt.bfloat16, tag="netc", name="netc")}
                pooled = pooledp.tile([128, 2, T], dt.bfloat16, tag="pooled",
                                      name="pooled")

                pTc = p2Tc = None
                for nt in range(T // 512):
                    if nt % 4 == 0:
                        pTc = tblp.tile([3, 2048], dt.bfloat16, tag="tbl",
                                        name="pTc")
                        p2Tc = tblp.tile([3, 2048], dt.bfloat16, tag="tbl",
                                         name="p2Tc")
                        nc.sync.dma_start(pTc[:], pT_d[:, nt * 512:(nt + 4) * 512])
                        nc.sync.dma_start(p2Tc[:], p2T_d[:, nt * 512:(nt + 4) * 512])
                    sl = slice(nt * 512, (nt + 1) * 512)
                    csl = slice((nt % 4) * 512, (nt % 4 + 1) * 512)
                    for m in range(2):
                        ps_g = psump.tile([H, 512], dt.float32, tag="ph", name="ps_g")
                        ps_c = psump.tile([H, 512], dt.float32, tag="po", name="ps_c")
                        nc.tensor.matmul(ps_g[:], wp_t[:, m * H:(m + 1) * H],
                                         pTc[:, csl], start=True, stop=True)
                        nc.tensor.matmul(ps_c[:], wp2_t[:, m * H:(m + 1) * H],
                                         p2Tc[:, csl], start=True, stop=True)
                        dg = net["g"][:, sl] if m == 0 else pooled[:, 0, sl]
                        dc = net["c"][:, sl] if m == 0 else pooled[:, 1, sl]
                        nc.scalar.activation(dg, ps_g[:],
                                             mybir.ActivationFunctionType.Copy)
                        nc.vector.tensor_tensor(out=dc, in0=dg, in1=ps_c[:],
                                                op=mybir.AluOpType.add)

                for s in ("g", "c"):
                    resblock(s, 0, net[s], pooled)

                for i in range(1, NB):
                    npm = make_net_pm(net)
                    for pl in range(NPLANES):
                        strip = stripp.tile([128, 2, WA[pl]], dt.bfloat16,
                                            tag="strip", name=f"strip{pl}")
                        stripB = stripp.tile([128, 2, WB[pl]], dt.bfloat16,
                                             tag="stripB", name=f"stripB{pl}")
                        sbuf_gather(strip[:], npm[:],
                                    smax_t[pl][:, :WA[pl] // 16], WA[pl])
                        sbuf_gather(stripB[:], npm[:],
                                    smax_t[pl][:, WA[pl] // 16:], WB[pl])
                        for j in range(1, len(WR[pl])):
                            wr = WR[pl][j]
                            o = int(OFF[pl][j])
                            src_ap = (strip[:, :, o:o + wr] if j < KS[pl] else
                                      stripB[:, :, o - int(OFF[pl][KS[pl]]):
                                             o - int(OFF[pl][KS[pl]]) + wr])
                            nc.vector.tensor_tensor(
                                out=strip[:, :, :wr], in0=strip[:, :, :wr],
                                in1=src_ap, op=mybir.AluOpType.max)
                        tbl = tblp.tile([128, N1[pl] // 128, 2 * H], dt.bfloat16,
                                        tag="tbl", name=f"tbl{pl}")
                        nc.sync.dma_start_transpose(tbl[:, :, 0:H],
                                                    strip[:, 0, :N1[pl]])
                        nc.sync.dma_start_transpose(tbl[:, :, H:2 * H],
                                                    strip[:, 1, :N1[pl]])
                        for q in range(2):
                            qsl = slice(q * 4096, (q + 1) * 4096)
                            exp = stripp.tile([128, 2, 4096], dt.bfloat16,
                                              tag="strip", name=f"exp{pl}_{q}")
                            sbuf_gather(exp[:], tbl[:],
                                        pidx_t[pl][:, q * 256:(q + 1) * 256], 4096)
                            if pl == 0:
                                nc.vector.tensor_copy(pooled[:, :, qsl], exp[:])
                            else:
                                nc.vector.tensor_tensor(
                                    out=pooled[:, :, qsl], in0=pooled[:, :, qsl],
                                    in1=exp[:], op=mybir.AluOpType.add)
                    for s in ("g", "c"):
                        resblock(s, i, net[s], pooled)

                # ---- mean stage ----
                npm_f = make_net_pm(net)
                for pl in range(NPLANES):
                    strip = stripp.tile([128, 2, WA[pl]], dt.bfloat16,
                                        tag="strip", name=f"mstrip{pl}")
                    stripB = stripp.tile([128, 2, WB[pl]], dt.bfloat16,
                                         tag="stripB", name=f"mstripB{pl}")
                    sbuf_gather(strip[:], npm_f[:],
                                ssum_t[pl][:, :WA[pl] // 16], WA[pl])
                    sbuf_gather(stripB[:], npm_f[:],
                                ssum_t[pl][:, WA[pl] // 16:], WB[pl])
                    acc = pooledp.tile([128, 2, N1[pl]], dt.float32, tag="pooled",
                                       name=f"acc{pl}")
                    nc.vector.tensor_copy(acc[:], strip[:, :, :N1[pl]])
                    for j in range(1, len(WR[pl])):
                        wr = WR[pl][j]
                        o = int(OFF[pl][j])
                        src_ap = (strip[:, :, o:o + wr] if j < KS[pl] else
                                  stripB[:, :, o - int(OFF[pl][KS[pl]]):
                                         o - int(OFF[pl][KS[pl]]) + wr])
                        nc.vector.tensor_tensor(
                            out=acc[:, :, :wr], in0=acc[:, :, :wr],
                            in1=src_ap, op=mybir.AluOpType.add)
                    accb = stripp.tile([128, 2, N1[pl]], dt.bfloat16, tag="strip",
                                       name=f"accb{pl}")
                    nc.vector.tensor_copy(accb[:], acc[:])
                    nch = N1[pl] // 128
                    sums = stripp.tile([128, nch, 2 * C], dt.float32, tag="sums",
                                       name=f"sums{pl}")
                    for k in range(nch):
                        pb = psump.tile([128, 2 * C], dt.float32, tag="ph", name="pb")
                        ksl = slice(k * 128, (k + 1) * 128)
                        nc.tensor.matmul(pb[:, :C], accb[:, 0, ksl], Wt["g"]["fcw"][:],
                                         start=True, stop=True)
                        nc.tensor.matmul(pb[:, C:], accb[:, 1, ksl], Wt["c"]["fcw"][:],
                                         start=True, stop=True)
                        nc.vector.tensor_copy(sums[:, k, :], pb[:])
                    nc.gpsimd.dma_scatter_add(
                        out_d[pl][:], sums[:], sbin_t[pl][:],
                        N1[pl], N1[pl], 2 * C, single_packet=False)

            if timing:
                chk_t = constp.tile([128, 128], dt.bfloat16)
                nc.vector.tensor_copy(chk_t[:], net["g"][:, :128])
                nc.sync.dma_start(chk_d[:], chk_t[:])

    nc.compile()

    # ---- per-core input maps ----
    in_maps = []
    for b in range(B):
        im = {
            "pT": np.ascontiguousarray(p[b].T).astype(BF),
            "p2T": np.ascontiguousarray(p2[b].T).astype(BF),
            "wp": wp.astype(BF), "wp2": wp2.astype(BF),
        }
        for s in ("g", "c"):
            sh = sh_host[s]
            w0pk = np.concatenate([sh["w0"][:, :H].transpose(1, 0, 2),
                                   sh["w0"][:, H:].transpose(1, 0, 2)], axis=2)
            wspk = np.concatenate([sh["ws"][:, :H].transpose(1, 0, 2),
                                   sh["ws"][:, H:].transpose(1, 0, 2)], axis=2)
            w1pk = sh["w1"].transpose(1, 0, 2)
            rb = np.zeros((H, NB, 2), F32)
            for i, (ba, bb) in enumerate(sh["relu_bias"]):
                rb[:, i, 0] = ba
                rb[:, i, 1] = bb
            im[f"{s}_w0"] = np.ascontiguousarray(w0pk).astype(BF)
            im[f"{s}_w1"] = np.ascontiguousarray(w1pk).astype(BF)
            im[f"{s}_ws"] = np.ascontiguousarray(wspk).astype(BF)
            im[f"{s}_rb"] = rb
            im[f"{s}_b0"] = np.ascontiguousarray(sh["b0"].T).astype(F32)
            im[f"{s}_fcw"] = fc_w[s].astype(BF)
        for pl in range(NPLANES):
            pr = preps[b][pl]
            k = KS[pl]
            padA = WA[pl] - int(OFF[pl][k])
            padB = WB[pl] - (W[pl] - int(OFF[pl][k]))
            smax = np.concatenate(
                [pr.round_ids(r, WR[pl][r - 1], zero_pad=False)
                 for r in range(1, k + 1)]
                + [np.full(padA, int(pr.members[0][0]), np.int64)]
                + [pr.round_ids(r, WR[pl][r - 1], zero_pad=False)
                   for r in range(k + 1, RMAX[pl] + 1)]
                + [np.full(padB, int(pr.members[0][0]), np.int64)])
            ssum = np.concatenate(
                [pr.round_ids(r, WR[pl][r - 1], zero_pad=True)
                 for r in range(1, k + 1)]
                + [np.full(padA, TZ, np.int64)]
                + [pr.round_ids(r, WR[pl][r - 1], zero_pad=True)
                   for r in range(k + 1, RMAX[pl] + 1)]
                + [np.full(padB, TZ, np.int64)])
            im[f"smax_{pl}"] = wrap_idxs(smax)
            im[f"ssum_{pl}"] = wrap_idxs(ssum)
            im[f"pidx_{pl}"] = wrap_idxs(pr.pidx)
            empty = np.where(pr.cnt == 0)[0]
            sb = np.full(N1[pl], int(empty[0]) if len(empty) else 0, np.int64)
            sb[:pr.n_occ] = pr.bins_sorted
            im[f"sbin_{pl}"] = wrap_idxs(sb)
        in_maps.append(im)

    return nc, in_maps, cvec


def kernel(**inputs):
    from concourse.bass_utils import run_bass_kernel_spmd

    preps = _prep(inputs)
    nc, in_maps, cvec = _build(inputs, preps, REPS=1, timing=False)
    res = run_bass_kernel_spmd(nc, in_maps, core_ids=list(range(B)))

    out = np.zeros((2 * NPLANES, B, C, R, R), F32)
    for b in range(B):
        for pl in range(NPLANES):
            grid = np.asarray(res.results[b][f"out_{pl}"], F32)
            pr = preps[b][pl]
            cnt = pr.cnt.astype(F32)
            for si, s in enumerate(("g", "c")):
                part = grid[:, si * C:(si + 1) * C]
                true_sums = part + cnt[:, None] * cvec[s][None, :]
                mean = true_sums / np.clip(cnt, 1.0, None)[:, None]
                mean[cnt == 0] = 0.0
                out[si * NPLANES + pl, b] = mean.T.reshape(C, R, R)
    return out


if __name__ == "__main__":
    import reference
    inputs = {k: np.asarray(v) for k, v in reference.setup_inputs().items()}
    result = kernel(**inputs)
    print("kernel output shape:", result.shape)


# revision 8
# speedup vs baseline: 122.3658x; 1.0009x over previous
"""TRN2 Bass kernel for nn_LocalPoolPointnetPPFusion (batch-parallel, 8 cores).

v3 = v1's dma_gather mechanism (7.6ns/idx Q7 desc-gen, engine-blocking) with
v2's op structure: both streams packed per token (512B payloads, one gather
serves g+c), all scatter-max rounds merged into ONE strip gather per plane
(DVE tensor_tensor maxes on strip segments, split A/B at a round boundary so
each gather fits the default SWDGE ring), expands split in 4096-idx halves.
~90 dyn-DMA ops and ~232k gathered indices per rep vs 832 ops / 518k in v1.
Measured: 3.22ms NTFF vs v1's 5.08ms.
"""
import sys
sys.path.insert(0, "/opt/trn_rl_repo")

import numpy as np
import ml_dtypes

BF = ml_dtypes.bfloat16
F32 = np.float32

B, T, H, C, R = 8, 8192, 128, 128, 128
NB = 5
NPLANES = 3
PLANE_COLS = ((0, 2), (0, 1), (1, 2))
TZ = T            # zero token: npm rank 64 is memset to 0


def compute_idx_lists(p_np):
    import jax
    import jax.numpy as jnp
    cpu = jax.devices("cpu")[0]
    out = []
    with jax.default_device(cpu):
        pj = jnp.asarray(p_np)
        for cols in PLANE_COLS:
            xy = pj[..., jnp.array(cols)] / (1.0 + 0.0 + 1e-3) + 0.5
            xy = jnp.clip(xy, 0.0, 1.0 - 1e-3)
            g = jnp.floor(xy * R).astype(jnp.int32)
            out.append(np.asarray(g[..., 0] + R * g[..., 1]))
    return out


def wrap_idxs(flat):
    flat = np.asarray(flat, np.int64)
    n = len(flat)
    assert n % 16 == 0
    a = flat.reshape(n // 16, 16).T.astype(np.int16)
    return np.tile(a, (8, 1))


def align(x, a):
    return (int(x) + a - 1) // a * a


class PlanePrep:
    def __init__(self, idx):
        self.idx = idx
        cnt = np.bincount(idx, minlength=R * R)
        self.cnt = cnt
        occ = np.where(cnt > 0)[0]
        order = np.argsort(-cnt[occ], kind="stable")
        self.bins_sorted = occ[order]
        self.n_occ = len(occ)
        self.occ_sorted = cnt[self.bins_sorted]
        sort_by_bin = np.argsort(idx, kind="stable")
        starts = np.searchsorted(idx[sort_by_bin], self.bins_sorted)
        self.members = [sort_by_bin[s:s + k] for s, k in zip(starts, self.occ_sorted)]
        slot_of_bin = np.full(R * R, -1, np.int64)
        slot_of_bin[self.bins_sorted] = np.arange(self.n_occ)
        self.pidx = slot_of_bin[idx]
        self.R_max = int(self.occ_sorted[0])
        self.n_r = [int((self.occ_sorted >= r).sum()) for r in range(1, self.R_max + 1)]

    def nr(self, r):
        return self.n_r[r - 1] if r <= self.R_max else 0

    def round_ids(self, r, width, zero_pad):
        ids = np.full(width, TZ if zero_pad else int(self.members[0][0]), np.int64)
        nr = self.nr(r)
        for s in range(min(nr, width)):
            ids[s] = self.members[s][r - 1]
        if not zero_pad:
            for s in range(nr, width):
                if s < self.n_occ:
                    ids[s] = self.members[s][0]
        return ids


def _prep(inputs):
    p = np.asarray(inputs["p"], F32)
    idx_lists = compute_idx_lists(p)
    return [[PlanePrep(idx_lists[pl][b]) for pl in range(NPLANES)] for b in range(B)]


def _build(inputs, preps, REPS=1, timing=False):
    import concourse.bacc as bacc
    import concourse.tile as tile
    from concourse import mybir

    p = np.asarray(inputs["p"], F32)
    p2 = np.asarray(inputs["p2"], F32)

    # ---- strip geometry (shared across batch) ----
    RMAX = [max(preps[b][pl].R_max for b in range(B)) for pl in range(NPLANES)]
    WR, OFF, N1, W, WG = [], [], [], [], []
    for pl in range(NPLANES):
        wr = [align(max(preps[b][pl].n_occ for b in range(B)), 128)]
        for r in range(2, RMAX[pl] + 1):
            wr.append(align(max(preps[b][pl].nr(r) for b in range(B)), 16))
        off = np.concatenate([[0], np.cumsum(wr)])
        WR.append(wr)
        OFF.append(off)
        N1.append(wr[0])
        W.append(int(off[-1]))
        WG.append(align(int(off[-1]), 128))
    # split point: first round boundary >= W/2 (part A holds rounds 1..KS-1)
    KS, WA, WB = [], [], []
    for pl in range(NPLANES):
        k = next(j for j in range(1, len(OFF[pl])) if OFF[pl][j] >= W[pl] / 2)
        KS.append(k)
        WA.append(align(int(OFF[pl][k]), 128))
        WB.append(align(W[pl] - int(OFF[pl][k]), 128))

    # ---- host-side weight/bias folding (identical to v1) ----
    def stream_host(pref, base_bias):
        w0 = np.asarray(inputs[f"{pref}_w0"], F32)
        b0 = np.asarray(inputs[f"{pref}_b0"], F32)
        w1 = np.asarray(inputs[f"{pref}_w1"], F32)
        b1 = np.asarray(inputs[f"{pref}_b1"], F32)
        ws = np.asarray(inputs[f"{pref}_ws"], F32)
        relu_bias = []
        Bp = base_bias
        for i in range(NB):
            if i == 0:
                bias_in = Bp
                relu_bias.append((bias_in[:H].copy(), bias_in[H:].copy()))
            else:
                bias_in = np.concatenate([Bp, 3.0 * Bp])
                relu_bias.append((Bp.copy(), 3.0 * Bp))
            Bp = b1[i] + bias_in @ ws[i]
        return dict(w0=w0, b0=b0, w1=w1, ws=ws, relu_bias=relu_bias, B_final=Bp)

    wp = np.asarray(inputs["wp"], F32)
    bp = np.asarray(inputs["bp"], F32)
    wp2 = np.asarray(inputs["wp2"], F32)
    bp2 = np.asarray(inputs["bp2"], F32)
    sh_host = {"g": stream_host("blk", bp.copy()), "c": stream_host("blkc", bp + bp2)}
    fc_w = {"g": np.asarray(inputs["fc_c_w"], F32),
            "c": np.asarray(inputs["fc_cc_w"], F32)}
    fc_b = {"g": np.asarray(inputs["fc_c_b"], F32),
            "c": np.asarray(inputs["fc_cc_b"], F32)}
    cvec = {s: sh_host[s]["B_final"] @ fc_w[s] + fc_b[s] for s in ("g", "c")}

    nc = bacc.Bacc("TRN2", target_bir_lowering=False, debug=False, num_devices=B)
    dt = mybir.dt

    def din(name, shape, dtype):
        return nc.dram_tensor(name, shape, dtype, kind="ExternalInput")

    pT_d = din("pT", [3, T], dt.bfloat16)
    p2T_d = din("p2T", [3, T], dt.bfloat16)
    wp_d = din("wp", [3, 2 * H], dt.bfloat16)
    wp2_d = din("wp2", [3, 2 * H], dt.bfloat16)
    wpk_d = {}
    for s in ("g", "c"):
        wpk_d[s] = dict(
            w0=din(f"{s}_w0", [H, NB, 2 * H], dt.bfloat16),
            w1=din(f"{s}_w1", [H, NB, H], dt.bfloat16),
            ws=din(f"{s}_ws", [H, NB, 2 * H], dt.bfloat16),
            rb=din(f"{s}_rb", [H, NB, 2], dt.float32),
            b0=din(f"{s}_b0", [H, NB], dt.float32),
            fcw=din(f"{s}_fcw", [H, C], dt.bfloat16),
        )
    smax_d = [din(f"smax_{pl}", [128, (WA[pl] + WB[pl]) // 16], dt.int16) for pl in range(NPLANES)]
    ssum_d = [din(f"ssum_{pl}", [128, (WA[pl] + WB[pl]) // 16], dt.int16) for pl in range(NPLANES)]
    pidx_d = [din(f"pidx_{pl}", [128, T // 16], dt.int16) for pl in range(NPLANES)]
    sbin_d = [din(f"sbin_{pl}", [128, N1[pl] // 16], dt.int16) for pl in range(NPLANES)]

    out_kind = "Internal" if timing else "ExternalOutput"
    out_d = {pl: nc.dram_tensor(f"out_{pl}", [R * R, 2 * C], dt.float32, kind=out_kind)
             for pl in range(NPLANES)}
    chk_d = nc.dram_tensor("chk", [128, 128], dt.bfloat16, kind="ExternalOutput") \
        if timing else None

    SI = {"g": 0, "c": 1}

    with tile.TileContext(nc) as tc:
        with tc.tile_pool(name="const", bufs=1) as constp, \
             tc.tile_pool(name="act", bufs=1) as actp, \
             tc.tile_pool(name="npm", bufs=1) as npmp, \
             tc.tile_pool(name="pooled", bufs=1) as pooledp, \
             tc.tile_pool(name="strip", bufs=1) as stripp, \
             tc.tile_pool(name="tbl", bufs=2) as tblp, \
             tc.tile_pool(name="small", bufs=2) as smallp, \
             tc.tile_pool(name="psum", bufs=2, space="PSUM") as psump:

            wp_t = constp.tile([3, 2 * H], dt.bfloat16)
            wp2_t = constp.tile([3, 2 * H], dt.bfloat16)
            nc.sync.dma_start(wp_t[:], wp_d[:])
            nc.sync.dma_start(wp2_t[:], wp2_d[:])
            Wt = {}
            for s in ("g", "c"):
                Wt[s] = dict(
                    w0=constp.tile([H, NB, 2 * H], dt.bfloat16, tag=f"{s}w0", name=f"{s}w0"),
                    w1=constp.tile([H, NB, H], dt.bfloat16, tag=f"{s}w1", name=f"{s}w1"),
                    ws=constp.tile([H, NB, 2 * H], dt.bfloat16, tag=f"{s}ws", name=f"{s}ws"),
                    rb=constp.tile([H, NB, 2], dt.float32, tag=f"{s}rb", name=f"{s}rb"),
                    b0=constp.tile([H, NB], dt.float32, tag=f"{s}b0", name=f"{s}b0"),
                    fcw=constp.tile([H, C], dt.bfloat16, tag=f"{s}fcw", name=f"{s}fcw"),
                )
                for k, t in Wt[s].items():
                    nc.sync.dma_start(t[:], wpk_d[s][k][:])
            smax_t, ssum_t, pidx_t, sbin_t = [], [], [], []
            for pl in range(NPLANES):
                smax_t.append(constp.tile([128, (WA[pl] + WB[pl]) // 16], dt.int16,
                                          tag=f"sm{pl}", name=f"smt{pl}"))
                ssum_t.append(constp.tile([128, (WA[pl] + WB[pl]) // 16], dt.int16,
                                          tag=f"ss{pl}", name=f"sst{pl}"))
                pidx_t.append(constp.tile([128, T // 16], dt.int16,
                                          tag=f"pi{pl}", name=f"pit{pl}"))
                sbin_t.append(constp.tile([128, N1[pl] // 16], dt.int16,
                                          tag=f"sb{pl}", name=f"sbt{pl}"))
                nc.sync.dma_start(smax_t[pl][:], smax_d[pl][:])
                nc.sync.dma_start(ssum_t[pl][:], ssum_d[pl][:])
                nc.sync.dma_start(pidx_t[pl][:], pidx_d[pl][:])
                nc.sync.dma_start(sbin_t[pl][:], sbin_d[pl][:])

            def sbuf_gather(dst_ap, src, idxs_ap, n):
                nc.gpsimd.dma_gather(
                    dst_ap, src, idxs_ap, n, n, 2 * H,
                    transpose=True, single_packet=False,
                    sbuf_tokens_per_rank=128,
                    sbuf_free_dim_per_rank=4 * H,
                )

            def make_net_pm(net):
                """Transpose both streams into npm [128, 65, 256]; rank 64 = 0."""
                npm = npmp.tile([128, 65, 2 * H], dt.bfloat16, tag="npm", name="npm")
                nc.vector.memset(npm[:, 64, :], 0.0)
                nc.sync.dma_start_transpose(npm[:, :64, 0:H], net["g"][:])
                nc.sync.dma_start_transpose(npm[:, :64, H:2 * H], net["c"][:])
                return npm

            def resblock(s, i, xa, pooled):
                """xa: [H, T] tile (in-place). pooled: [128, 2, T] tile."""
                w = Wt[s]
                si = SI[s]
                ba_ap = w["rb"][:, i, 0:1]
                bb_ap = w["rb"][:, i, 1:2]
                for nt in range(T // 512):
                    sl = slice(nt * 512, (nt + 1) * 512)
                    xb = pooled[:, si, sl]
                    ra = smallp.tile([H, 512], dt.bfloat16, tag="ra", name="ra")
                    rb_ = smallp.tile([H, 512], dt.bfloat16, tag="rb", name="rb")
                    nc.vector.tensor_scalar(out=ra[:], in0=xa[:, sl], scalar1=ba_ap,
                                            scalar2=0.0, op0=mybir.AluOpType.add,
                                            op1=mybir.AluOpType.max)
                    nc.vector.tensor_scalar(out=rb_[:], in0=xb, scalar1=bb_ap,
                                            scalar2=0.0, op0=mybir.AluOpType.add,
                                            op1=mybir.AluOpType.max)
                    ph = psump.tile([H, 512], dt.float32, tag="ph", name="ph")
                    nc.tensor.matmul(ph[:], w["w0"][:, i, :H], ra[:],
                                     start=True, stop=False)
                    nc.tensor.matmul(ph[:], w["w0"][:, i, H:], rb_[:],
                                     start=False, stop=True)
                    h = smallp.tile([H, 512], dt.bfloat16, tag="h", name="h")
                    nc.scalar.activation(h[:], ph[:], mybir.ActivationFunctionType.Relu,
                                         bias=w["b0"][:, i:i + 1], scale=1.0)
                    po = psump.tile([H, 512], dt.float32, tag="po", name="po")
                    nc.tensor.matmul(po[:], w["w1"][:, i, :], h[:],
                                     start=True, stop=False)
                    nc.tensor.matmul(po[:], w["ws"][:, i, :H], xa[:, sl],
                                     start=False, stop=False)
                    nc.tensor.matmul(po[:], w["ws"][:, i, H:], xb,
                                     start=False, stop=True)
                    nc.scalar.activation(xa[:, sl], po[:],
                                         mybir.ActivationFunctionType.Copy)

            # ---------------- schedule ----------------
            for rep in range(REPS):
                net = {"g": actp.tile([H, T], dt.bfloat16, tag="netg", name="netg"),
                       "c": actp.tile([H, T], dt.bfloat16, tag="netc", name="netc")}
                pooled = pooledp.tile([128, 2, T], dt.bfloat16, tag="pooled",
                                      name="pooled")

                pTc = p2Tc = None
                for nt in range(T // 512):
                    if nt % 4 == 0:
                        pTc = tblp.tile([3, 2048], dt.bfloat16, tag="tbl",
                                        name="pTc")
                        p2Tc = tblp.tile([3, 2048], dt.bfloat16, tag="tbl",
                                         name="p2Tc")
                        nc.sync.dma_start(pTc[:], pT_d[:, nt * 512:(nt + 4) * 512])
                        nc.sync.dma_start(p2Tc[:], p2T_d[:, nt * 512:(nt + 4) * 512])
                    sl = slice(nt * 512, (nt + 1) * 512)
                    csl = slice((nt % 4) * 512, (nt % 4 + 1) * 512)
                    for m in range(2):
                        ps_g = psump.tile([H, 512], dt.float32, tag="ph", name="ps_g")
                        ps_c = psump.tile([H, 512], dt.float32, tag="po", name="ps_c")
                        nc.tensor.matmul(ps_g[:], wp_t[:, m * H:(m + 1) * H],
                                         pTc[:, csl], start=True, stop=True)
                        nc.tensor.matmul(ps_c[:], wp2_t[:, m * H:(m + 1) * H],
                                         p2Tc[:, csl], start=True, stop=True)
                        dg = net["g"][:, sl] if m == 0 else pooled[:, 0, sl]
                        dc = net["c"][:, sl] if m == 0 else pooled[:, 1, sl]
                        nc.scalar.activation(dg, ps_g[:],
                                             mybir.ActivationFunctionType.Copy)
                        nc.vector.tensor_tensor(out=dc, in0=dg, in1=ps_c[:],
                                                op=mybir.AluOpType.add)

                for s in ("g", "c"):
                    resblock(s, 0, net[s], pooled)

                for i in range(1, NB):
                    npm = make_net_pm(net)
                    for pl in range(NPLANES):
                        strip = stripp.tile([128, 2, WA[pl]], dt.bfloat16,
                                            tag="strip", name=f"strip{pl}")
                        stripB = stripp.tile([128, 2, WB[pl]], dt.bfloat16,
                                             tag="stripB", name=f"stripB{pl}")
                        sbuf_gather(strip[:], npm[:],
                                    smax_t[pl][:, :WA[pl] // 16], WA[pl])
                        sbuf_gather(stripB[:], npm[:],
                                    smax_t[pl][:, WA[pl] // 16:], WB[pl])
                        for j in range(1, len(WR[pl])):
                            wr = WR[pl][j]
                            o = int(OFF[pl][j])
                            src_ap = (strip[:, :, o:o + wr] if j < KS[pl] else
                                      stripB[:, :, o - int(OFF[pl][KS[pl]]):
                                             o - int(OFF[pl][KS[pl]]) + wr])
                            nc.vector.tensor_tensor(
                                out=strip[:, :, :wr], in0=strip[:, :, :wr],
                                in1=src_ap, op=mybir.AluOpType.max)
                        tbl = tblp.tile([128, N1[pl] // 128, 2 * H], dt.bfloat16,
                                        tag="tbl", name=f"tbl{pl}")
                        nc.sync.dma_start_transpose(tbl[:, :, 0:H],
                                                    strip[:, 0, :N1[pl]])
                        nc.sync.dma_start_transpose(tbl[:, :, H:2 * H],
                                                    strip[:, 1, :N1[pl]])
                        for q in range(2):
                            qsl = slice(q * 4096, (q + 1) * 4096)
                            exp = stripp.tile([128, 2, 4096], dt.bfloat16,
                                              tag="strip", name=f"exp{pl}_{q}")
                            sbuf_gather(exp[:], tbl[:],
                                        pidx_t[pl][:, q * 256:(q + 1) * 256], 4096)
                            if pl == 0:
                                nc.vector.tensor_copy(pooled[:, :, qsl], exp[:])
                            else:
                                nc.vector.tensor_tensor(
                                    out=pooled[:, :, qsl], in0=pooled[:, :, qsl],
                                    in1=exp[:], op=mybir.AluOpType.add)
                    for s in ("g", "c"):
                        resblock(s, i, net[s], pooled)

                # ---- mean stage ----
                npm_f = make_net_pm(net)
                for pl in range(NPLANES):
                    strip = stripp.tile([128, 2, WA[pl]], dt.bfloat16,
                                        tag="strip", name=f"mstrip{pl}")
                    stripB = stripp.tile([128, 2, WB[pl]], dt.bfloat16,
                                         tag="stripB", name=f"mstripB{pl}")
                    sbuf_gather(strip[:], npm_f[:],
                                ssum_t[pl][:, :WA[pl] // 16], WA[pl])
                    sbuf_gather(stripB[:], npm_f[:],
                                ssum_t[pl][:, WA[pl] // 16:], WB[pl])
                    acc = pooledp.tile([128, 2, N1[pl]], dt.float32, tag="pooled",
                                       name=f"acc{pl}")
                    nc.vector.tensor_copy(acc[:], strip[:, :, :N1[pl]])
                    for j in range(1, len(WR[pl])):
                        wr = WR[pl][j]
                        o = int(OFF[pl][j])
                        src_ap = (strip[:, :, o:o + wr] if j < KS[pl] else
                                  stripB[:, :, o - int(OFF[pl][KS[pl]]):
                                         o - int(OFF[pl][KS[pl]]) + wr])
                        nc.vector.tensor_tensor(
                            out=acc[:, :, :wr], in0=acc[:, :, :wr],
                            in1=src_ap, op=mybir.AluOpType.add)
                    accb = stripp.tile([128, 2, N1[pl]], dt.bfloat16, tag="strip",
                                       name=f"accb{pl}")
                    nc.vector.tensor_copy(accb[:], acc[:])
                    nch = N1[pl] // 128
                    sums = stripp.tile([128, nch, 2 * C], dt.float32, tag="sums",
                                       name=f"sums{pl}")
                    for k in range(nch):
                        pb = psump.tile([128, 2 * C], dt.float32, tag="ph", name="pb")
                        ksl = slice(k * 128, (k + 1) * 128)
                        nc.tensor.matmul(pb[:, :C], accb[:, 0, ksl], Wt["g"]["fcw"][:],
                                         start=True, stop=True)
                        nc.tensor.matmul(pb[:, C:], accb[:, 1, ksl], Wt["c"]["fcw"][:],
                                         start=True, stop=True)
                        nc.vector.tensor_copy(sums[:, k, :], pb[:])
                    nc.gpsimd.dma_scatter_add(
                        out_d[pl][:], sums[:], sbin_t[pl][:],
                        N1[pl], N1[pl], 2 * C, single_packet=False)

            if timing:
                chk_t = constp.tile([128, 128], dt.bfloat16)
                nc.vector.tensor_copy(chk_t[:], net["g"][:, :128])
                nc.sync.dma_start(chk_d[:], chk_t[:])

    nc.compile()

    # ---- per-core input maps ----
    in_maps = []
    for b in range(B):
        im = {
            "pT": np.ascontiguousarray(p[b].T).astype(BF),
            "p2T": np.ascontiguousarray(p2[b].T).astype(BF),
            "wp": wp.astype(BF), "wp2": wp2.astype(BF),
        }
        for s in ("g", "c"):
            sh = sh_host[s]
            w0pk = np.concatenate([sh["w0"][:, :H].transpose(1, 0, 2),
                                   sh["w0"][:, H:].transpose(1, 0, 2)], axis=2)
            wspk = np.concatenate([sh["ws"][:, :H].transpose(1, 0, 2),
                                   sh["ws"][:, H:].transpose(1, 0, 2)], axis=2)
            w1pk = sh["w1"].transpose(1, 0, 2)
            rb = np.zeros((H, NB, 2), F32)
            for i, (ba, bb) in enumerate(sh["relu_bias"]):
                rb[:, i, 0] = ba
                rb[:, i, 1] = bb
            im[f"{s}_w0"] = np.ascontiguousarray(w0pk).astype(BF)
            im[f"{s}_w1"] = np.ascontiguousarray(w1pk).astype(BF)
            im[f"{s}_ws"] = np.ascontiguousarray(wspk).astype(BF)
            im[f"{s}_rb"] = rb
            im[f"{s}_b0"] = np.ascontiguousarray(sh["b0"].T).astype(F32)
            im[f"{s}_fcw"] = fc_w[s].astype(BF)
        for pl in range(NPLANES):
            pr = preps[b][pl]
            k = KS[pl]
            padA = WA[pl] - int(OFF[pl][k])
            padB = WB[pl] - (W[pl] - int(OFF[pl][k]))
            smax = np.concatenate(
                [pr.round_ids(r, WR[pl][r - 1], zero_pad=False)
                 for r in range(1, k + 1)]
                + [np.full(padA, int(pr.members[0][0]), np.int64)]
                + [pr.round_ids(r, WR[pl][r - 1], zero_pad=False)
                   for r in range(k + 1, RMAX[pl] + 1)]
                + [np.full(padB, int(pr.members[0][0]), np.int64)])
            ssum = np.concatenate(
                [pr.round_ids(r, WR[pl][r - 1], zero_pad=True)
                 for r in range(1, k + 1)]
                + [np.full(padA, TZ, np.int64)]
                + [pr.round_ids(r, WR[pl][r - 1], zero_pad=True)
                   for r in range(k + 1, RMAX[pl] + 1)]
                + [np.full(padB, TZ, np.int64)])
            im[f"smax_{pl}"] = wrap_idxs(smax)
            im[f"ssum_{pl}"] = wrap_idxs(ssum)
            im[f"pidx_{pl}"] = wrap_idxs(pr.pidx)
            empty = np.where(pr.cnt == 0)[0]
            sb = np.full(N1[pl], int(empty[0]) if len(empty) else 0, np.int64)
            sb[:pr.n_occ] = pr.bins_sorted
            im[f"sbin_{pl}"] = wrap_idxs(sb)
        in_maps.append(im)

    return nc, in_maps, cvec


def kernel(**inputs):
    from concourse.bass_utils import run_bass_kernel_spmd

    preps = _prep(inputs)
    nc, in_maps, cvec = _build(inputs, preps, REPS=1, timing=False)
    res = run_bass_kernel_spmd(nc, in_maps, core_ids=list(range(B)))

    out = np.zeros((2 * NPLANES, B, C, R, R), F32)
    for b in range(B):
        for pl in range(NPLANES):
            grid = np.asarray(res.results[b][f"out_{pl}"], F32)
            pr = preps[b][pl]
            cnt = pr.cnt.astype(F32)
            for si, s in enumerate(("g", "c")):
                part = grid[:, si * C:(si + 1) * C]
                true_sums = part + cnt[:, None] * cvec[s][None, :]
                mean = true_sums / np.clip(cnt, 1.0, None)[:, None]
                mean[cnt == 0] = 0.0
                out[si * NPLANES + pl, b] = mean.T.reshape(C, R, R)
    return out


if __name__ == "__main__":
    import reference
    inputs = {k: np.asarray(v) for k, v in reference.setup_inputs().items()}
    result = kernel(**inputs)
    print("kernel output shape:", result.shape)
